# revision 29
# baseline (speedup 1.0000x reference)
"""Trainium2 fused Bass kernel for JetGNN (2-layer SAGEConv + global mean pool).

Strategy (8 NeuronCores, graph-aligned node sharding, single fused NEFF):
  - Host: graph-aligned core boundaries (batch sorted); per-core nodes sorted
    by in-degree (desc) so per-tile max degree K[t] stays small; one padded
    neighbor table (positions in the AllGathered [8*S] layout) serves both
    layers; invdeg / valid-mask / local-graph-id tables per tile.
  - Device (SPMD x8, one NEFF):
      phase 0: AllGather own x shard (bf16)  -> x_full   [8S, 32]
      phase 1: per dst-tile indirect gather of x_full[nbr] -> strided DVE
               neighbor-sum -> mean -> PE transpose -> W1 matmuls (PSUM acc)
               -> +b1, mask, ReLU -> h1 tile (bf16) -> h1_shard DRAM
               (+ PE transpose into resident selfT2 for the layer-2 self path)
      AllGather h1_shard -> h1_full [8S, 64] (bf16)
      phase 2: same message passing vs h1_full; ReLU -> h2 (bf16); global
               pooling via one-hot (is_equal vs iota) matmuls accumulated in
               SBUF -> pool sums [640, 64] per core.
  - Host: divide by graph node counts, final 64->2 linear (0.3% of FLOPs).

Timing: a persistent jit(shard_map) runner (the same _bass_exec_p machinery
run_bass_kernel_spmd uses under axon) compiles once, keeps inputs on device,
and reports steady-state execution time (block_until_ready, best of reps).
"""

import math
import time

import numpy as np
import ml_dtypes

import concourse.bass as bass
import concourse.tile as tile
import concourse.mybir as mybir
from concourse import bacc

N_NODES = 200000
N_GRAPHS = 4000
N_CORES = 8
IN_CH = 32
HID = 64
P = 128
POOL_SLOTS = 640  # per-core graph slots (~500 graphs/core); last slot = dump
DUMP_SLOT = POOL_SLOTS - 1
POOL_CHUNKS = POOL_SLOTS // P

f32 = mybir.dt.float32
bf16 = mybir.dt.bfloat16
i32 = mybir.dt.int32
bfl = ml_dtypes.bfloat16


# ----------------------------------------------------------------- host prep
def _prep(edge_index, batch, n_nodes, n_graphs, n_cores):
    """Vectorized host prep. Returns layout + per-core tables."""
    src = np.asarray(edge_index[0]).astype(np.int64)
    dst = np.asarray(edge_index[1]).astype(np.int64)
    batch = np.asarray(batch).astype(np.int64)
    deg = np.bincount(dst, minlength=n_nodes).astype(np.int64)

    # CSR by dst
    order = np.argsort(dst, kind="stable")
    src_sorted = src[order]
    rowptr = np.zeros(n_nodes + 1, dtype=np.int64)
    np.cumsum(deg, out=rowptr[1:])

    # graph-aligned core boundaries (batch is sorted by graph id)
    gcnt = np.bincount(batch, minlength=n_graphs)
    gends = np.cumsum(gcnt)
    targets = (np.arange(1, n_cores) * n_nodes) // n_cores
    gb = np.searchsorted(gends, targets)
    graph_bounds = np.concatenate([[0], gb + 1, [n_graphs]])
    node_bounds = np.concatenate([[0], gends[graph_bounds[1:-1] - 1], [n_nodes]])

    cores = []
    for c in range(n_cores):
        lo, hi = int(node_bounds[c]), int(node_bounds[c + 1])
        perm = np.argsort(-deg[lo:hi], kind="stable")
        ids = np.arange(lo, hi)[perm]
        glo, ghi = int(graph_bounds[c]), int(graph_bounds[c + 1])
        assert ghi - glo <= DUMP_SLOT
        cores.append(dict(ids=ids, glo=glo, ghi=ghi, n=hi - lo))

    T = max(1, math.ceil(max(ci["n"] for ci in cores) / P))
    S = T * P
    # per-tile K: max over cores of max degree within the tile
    K = np.ones(T, dtype=np.int64)
    for ci in cores:
        d = np.pad(deg[ci["ids"]], (0, S - ci["n"]))
        K = np.maximum(K, d.reshape(T, P).max(axis=1))
    offs = np.zeros(T + 1, dtype=np.int64)
    np.cumsum(K, out=offs[1:])
    C = int(offs[-1])

    # global position of each node in the AllGathered [n_cores*S] layout
    global_pos = np.empty(n_nodes, dtype=np.int64)
    for c, ci in enumerate(cores):
        global_pos[ci["ids"]] = c * S + np.arange(ci["n"])
    pad_core = next(c for c, ci in enumerate(cores) if ci["n"] < S)
    pad_slot = pad_core * S + cores[pad_core]["n"]  # guaranteed all-zero row

    for c, ci in enumerate(cores):
        ids, n = ci["ids"], ci["n"]
        deg_c = deg[ids]
        tot = int(deg_c.sum())
        # edge-wise coordinates: node at position pos -> tile t, row p
        cum0 = np.zeros(n, dtype=np.int64)
        np.cumsum(deg_c[:-1], out=cum0[1:])
        e_node = np.repeat(np.arange(n), deg_c)  # position of dst node
        j = np.arange(tot) - np.repeat(cum0, deg_c)  # slot within nbr list
        e_idx = np.repeat(rowptr[ids], deg_c) + j  # CSR edge index
        t_e = e_node // P
        p_e = e_node % P
        col = offs[t_e] + j
        nbr = np.full(P * C, pad_slot, dtype=np.int32)
        nbr[p_e * C + col] = global_pos[src_sorted[e_idx]].astype(np.int32)

        def _tileize(vals, pad_val, dtype):
            a = np.full(S, pad_val, dtype=dtype)
            a[:n] = vals
            return np.ascontiguousarray(a.reshape(T, P).T)

        invdeg = _tileize(1.0 / np.maximum(deg_c, 1), 0.0, np.float32)
        maskf = _tileize(np.ones(n), 0.0, np.float32)
        localgf = _tileize(batch[ids] - ci["glo"], DUMP_SLOT, np.float32)
        ci.update(nbr=nbr.reshape(P, C), invdeg=invdeg, maskf=maskf,
                  localgf=localgf)

    return dict(cores=cores, T=T, S=S, K=K.tolist(), offs=offs.tolist(), C=C,
                node_bounds=node_bounds, graph_bounds=graph_bounds,
                gcnt=gcnt, batch=batch)


# ----------------------------------------------------- edge-major host prep
GT = 4  # dst tiles per group (512 slots -> one 2KB PSUM bank at f32)
GATHER_CHUNK = 512  # idxs per dma_gather (best 4-queue balance measured)
SENT = 4096.0  # dstid sentinel for pad edges (matches no iota column)


def _prep_eg(edge_index, batch, n_nodes, n_graphs, n_cores):
    """Edge-major prep for the dma_gather kernel.

    Edges are bucketed per (dst-tile-group, source-core window), sorted, and
    padded to a single structure shared by all cores (max over cores), so the
    SPMD program is uniform. Indices are window-local (int16) positions into
    the AllGathered [n_cores*S] node table.
    """
    src = np.asarray(edge_index[0]).astype(np.int64)
    dst = np.asarray(edge_index[1]).astype(np.int64)
    batch = np.asarray(batch).astype(np.int64)
    deg = np.bincount(dst, minlength=n_nodes).astype(np.int64)

    gcnt = np.bincount(batch, minlength=n_graphs)
    gends = np.cumsum(gcnt)
    targets = (np.arange(1, n_cores) * n_nodes) // n_cores
    gb = np.searchsorted(gends, targets)
    graph_bounds = np.concatenate([[0], gb + 1, [n_graphs]])
    node_bounds = np.concatenate([[0], gends[graph_bounds[1:-1] - 1],
                                  [n_nodes]])

    cores = []
    for c in range(n_cores):
        lo, hi = int(node_bounds[c]), int(node_bounds[c + 1])
        perm = np.argsort(-deg[lo:hi], kind="stable")
        ids = np.arange(lo, hi)[perm]
        glo, ghi = int(graph_bounds[c]), int(graph_bounds[c + 1])
        assert ghi - glo <= DUMP_SLOT
        slot = np.empty(hi - lo, dtype=np.int64)
        slot[perm] = np.arange(hi - lo)
        cores.append(dict(ids=ids, glo=glo, ghi=ghi, n=hi - lo, lo=lo, hi=hi,
                          slot=slot))

    max_n = max(ci["n"] for ci in cores)
    T = math.ceil((max_n + 1) / P)  # +1: every core keeps a zero pad row
    T = ((T + GT - 1) // GT) * GT
    S = T * P
    GN = T // GT
    GS = GT * P  # slots per group

    global_pos = np.empty(n_nodes, dtype=np.int64)
    for c, ci in enumerate(cores):
        global_pos[ci["ids"]] = c * S + np.arange(ci["n"])

    # per-core edge buckets -> uniform (max over cores) segment sizes
    cnts = np.zeros((n_cores, GN, n_cores), dtype=np.int64)
    percore = []
    for c, ci in enumerate(cores):
        emask = (dst >= ci["lo"]) & (dst < ci["hi"])
        ed, es = dst[emask], src[emask]
        dslot = ci["slot"][ed - ci["lo"]]
        sg = global_pos[es]
        g_e = dslot // GS
        w_e = sg // S
        order = np.lexsort((dslot, w_e, g_e))
        percore.append(dict(
            g=g_e[order], w=w_e[order],
            loc=(sg[order] % S).astype(np.int16),
            did=(dslot[order] - g_e[order] * GS).astype(np.float32),
            inv=(1.0 / deg[ed[order]]).astype(np.float32)))
        np.add.at(cnts[c], (g_e, w_e), 1)

    Lgw = ((cnts.max(axis=0) + P - 1) // P) * P  # [GN, n_cores]
    # guarantee >= 1 block per group (isolated/pad-only groups)
    empty_g = Lgw.sum(axis=1) == 0
    Lgw[empty_g, 0] = P
    NB = (Lgw.sum(axis=1) // P).astype(np.int64)  # blocks per group
    POS = NB * P
    seg_base = np.zeros((GN, n_cores), dtype=np.int64)
    pos_base = np.zeros(GN + 1, dtype=np.int64)
    for g in range(GN):
        pos_base[g + 1] = pos_base[g] + POS[g]
        seg_base[g] = pos_base[g] + np.concatenate(
            [[0], np.cumsum(Lgw[g][:-1])])
    POSTOT = int(pos_base[-1])
    NBTOT = POSTOT // P
    # static segments for codegen: per group, (window, n_blocks, idx col/16,
    # out block offset) — chunked to <= 2048 idxs
    segs = []
    for g in range(GN):
        sg_list = []
        for w in range(n_cores):
            L = int(Lgw[g][w])
            off = int(seg_base[g][w] - pos_base[g])
            while L > 0:
                chunk = min(L, GATHER_CHUNK)
                sg_list.append((w, chunk // P, off // 16, off // P))
                off += chunk
                L -= chunk
        segs.append(sg_list)

    for c, ci in enumerate(cores):
        pc = percore[c]
        cell = pc["g"] * n_cores + pc["w"]
        cell_counts = np.bincount(cell, minlength=GN * n_cores)
        cell_start = np.concatenate([[0], np.cumsum(cell_counts)[:-1]])
        rank = np.arange(len(cell)) - cell_start[cell]
        tgt = seg_base.reshape(-1)[cell] + rank
        idxf = np.full(POSTOT, S - 1, dtype=np.int16)
        didf = np.full(POSTOT, SENT, dtype=np.float32)
        invf = np.zeros(POSTOT, dtype=np.float32)
        idxf[tgt] = pc["loc"]
        didf[tgt] = pc["did"]
        invf[tgt] = pc["inv"]
        ci["idx"] = np.tile(np.ascontiguousarray(
            idxf.reshape(-1, 16).T), (n_cores, 1))
        ci["dstid"] = np.ascontiguousarray(didf.reshape(-1, P).T)
        ci["invde"] = np.ascontiguousarray(invf.reshape(-1, P).T)
        nbtot = POSTOT // P
        dinv = np.empty((P, 2 * nbtot), np.float32)
        for g in range(GN):
            bb = pos_base[g] // P
            nb = int(NB[g])
            dinv[:, 2 * bb:2 * bb + nb] = ci["dstid"][:, bb:bb + nb]
            dinv[:, 2 * bb + nb:2 * (bb + nb)] = ci["invde"][:, bb:bb + nb]
        ci["dinv"] = dinv
        lg = np.full(S, DUMP_SLOT, dtype=np.float32)
        lg[:ci["n"]] = batch[ci["ids"]] - ci["glo"]
        ci["localgf"] = np.ascontiguousarray(lg.reshape(T, P).T)

    # in-group slice offsets for per-group table streams
    return dict(cores=cores, T=T, S=S, GN=GN, NB=NB.tolist(),
                POS=POS.tolist(), pos_base=pos_base.tolist(), segs=segs,
                gcnt=gcnt, batch=batch)


# ------------------------------------------------- edge-major kernel builder
def _build_eg(T, S, GN, NB, POS, pos_base, segs, n_cores, no_gather=False,
              with_lib=True, no_cc=False, no_pool=False, no_ind=False,
              no_bmm=False, scratch=None, nq=4):
    from concourse.library_config import mlp as mlp_lib

    AS = n_cores * S
    XCOL = 128  # gathered row width (256B at bf16, dma_gather granularity)
    CC = 64  # compact row width for DRAM tables / collectives
    NBTOT = pos_base[-1] // P
    kw = dict(dynamic_dma_scratch_size=scratch) if scratch else {}
    nc = bacc.Bacc("TRN2", target_bir_lowering=False, debug=False,
                   enable_asserts=False, num_devices=n_cores,
                   num_swdge_queues=nq, **kw)
    i16 = mybir.dt.int16
    qrr = [0]
    xs = nc.dram_tensor("xs", [S, CC], bf16, kind="ExternalInput").ap()
    selfT1 = nc.dram_tensor("selfT1", [IN_CH, S], bf16,
                            kind="ExternalInput").ap()
    idx = nc.dram_tensor("idx", [P, pos_base[-1] // 16], i16,
                         kind="ExternalInput").ap()
    dinv = nc.dram_tensor("dinv", [P, 2 * NBTOT], f32,
                          kind="ExternalInput").ap()
    localgf = nc.dram_tensor("localgf", [P, T], f32, kind="ExternalInput").ap()
    iota = nc.dram_tensor("iota", [P, POOL_SLOTS], f32,
                          kind="ExternalInput").ap()
    identb = nc.dram_tensor("identb", [P, P], bf16, kind="ExternalInput").ap()
    w1big = nc.dram_tensor("w1big", [2 * IN_CH + 1, HID], bf16,
                           kind="ExternalInput").ap()
    w2big = nc.dram_tensor("w2big", [2 * HID, HID], bf16,
                           kind="ExternalInput").ap()
    b2rep = nc.dram_tensor("b2rep", [P, HID], f32, kind="ExternalInput").ap()
    pool = nc.dram_tensor("pool", [HID, POOL_SLOTS], f32,
                          kind="ExternalOutput").ap()

    rg = [list(range(n_cores))]
    with tile.TileContext(nc) as tc:
        if with_lib:
            nc.gpsimd.load_library(mlp_lib)
        with tc.tile_pool(name="dramp", bufs=1, space="DRAM") as dpool, \
             tc.tile_pool(name="resident", bufs=1) as rpool:
            x_shard = dpool.tile([S, CC], bf16, tag="x_shard")
            x_cat = dpool.tile([AS, CC], bf16, addr_space="Shared",
                               tag="x_cat")
            x_full = dpool.tile([AS, XCOL], bf16, tag="x_full")
            h1_shard = dpool.tile([S, CC], bf16, tag="h1_shard")
            h1_cat = dpool.tile([AS, CC], bf16, addr_space="Shared",
                                tag="h1_cat")
            h1_full = dpool.tile([AS, XCOL], bf16, tag="h1_full")

            nc.gpsimd.dma_start(x_shard[:], xs[:])
            if no_cc:
                nc.gpsimd.dma_start(x_cat[:S, :], x_shard[:])
            else:
                nc.gpsimd.collective_compute(
                    "AllGather", mybir.AluOpType.bypass, replica_groups=rg,
                    ins=[x_shard[:].opt()], outs=[x_cat[:].opt()])
            # spread compact rows to 256B pitch for dma_gather
            # (split: DMA AP dim counts are 16-bit)
            for w in range(n_cores):
                nc.sync.dma_start(x_full[w * S:(w + 1) * S, :CC],
                                  x_cat[w * S:(w + 1) * S, :])

            stacked1 = rpool.tile([2 * IN_CH + 1, T * P], bf16,
                                  tag="stacked1")
            nc.sync.dma_start(stacked1[IN_CH:2 * IN_CH, :], selfT1[:])
            nc.vector.memset(stacked1[2 * IN_CH:2 * IN_CH + 1, :], 1.0)
            stacked2 = rpool.tile([2 * HID, T * P], bf16, tag="stacked2")
            localgf_sb = rpool.tile([P, T], f32, tag="localgf")
            nc.sync.dma_start(localgf_sb[:], localgf[:])
            iota_sb = rpool.tile([P, POOL_SLOTS], f32, tag="iota")
            nc.sync.dma_start(iota_sb[:], iota[:])
            identb_sb = rpool.tile([P, P], bf16, tag="identb")
            nc.sync.dma_start(identb_sb[:], identb[:])
            w1big_sb = rpool.tile([2 * IN_CH + 1, HID], bf16, tag="w1big")
            nc.sync.dma_start(w1big_sb[:], w1big[:])
            w2big_sb = rpool.tile([2 * HID, HID], bf16, tag="w2big")
            nc.sync.dma_start(w2big_sb[:], w2big[:])
            b2rep_sb = rpool.tile([P, HID], f32, tag="b2rep")
            nc.sync.dma_start(b2rep_sb[:], b2rep[:])
            accT_sb = rpool.tile([HID, POOL_SLOTS], f32, tag="accT")
            nc.vector.memset(accT_sb[:], 0.0)
            zrow = rpool.tile([1, CC], bf16, tag="zrow")
            nc.vector.memset(zrow[:], 0.0)

            def gather_group(g, src_full, gpool, spool):
                nb, ps = NB[g], POS[g]
                bb = pos_base[g] // P
                idx_g = spool.tile([P, ps // 16], i16, tag="idxg")
                nc.sync.dma_start(
                    idx_g[:], idx[:, pos_base[g] // 16:pos_base[g + 1] // 16])
                dinv_g = spool.tile([P, 2 * nb], f32, tag="dinvg")
                nc.sync.dma_start(dinv_g[:], dinv[:, 2 * bb:2 * (bb + nb)])
                did_g = dinv_g[:, :nb]
                inv_g = dinv_g[:, nb:]
                xe = gpool.tile([P, nb * XCOL], bf16, tag="xe")
                if no_gather:
                    nc.vector.memset(xe[:, :XCOL], 0.0)
                if not no_gather:
                    for (w, blocks, icol, boff) in segs[g]:
                        L = blocks * P
                        qrr[0] = (qrr[0] + 1) % nq
                        nc.gpsimd.dma_gather(
                            xe[:, boff * XCOL:(boff + blocks) * XCOL].rearrange(
                                "p (b e) -> p b e", e=XCOL),
                            src_full[w * S:(w + 1) * S, :],
                            idx_g[:, icol:icol + L // 16], L, L, XCOL,
                            queue_num=qrr[0])
                return xe, did_g, inv_g

            def seg_sum(nb, xe, did_g, inv_g, F, ps_a, wpool, tag):
                acc_ps = ps_a.tile([F, GT * P], f32, tag="accp" + tag)
                ind0 = None
                if no_ind:
                    ind0 = wpool.tile([P, GT * P], bf16, tag="ind" + tag)
                    nc.vector.memset(ind0[:, :1], 0.0)
                for b in range(nb):
                    if no_ind:
                        ind = ind0
                    else:
                        ind = wpool.tile([P, GT * P], bf16, tag="ind" + tag)
                        nc.vector.tensor_scalar(
                            ind[:], iota_sb[:, :GT * P], did_g[:, b:b + 1],
                            inv_g[:, b:b + 1],
                            op0=mybir.AluOpType.is_equal,
                            op1=mybir.AluOpType.mult)
                    if no_bmm and 0 < b < nb - 1:
                        continue
                    nc.tensor.matmul(
                        acc_ps[:], lhsT=xe[:, b * XCOL:b * XCOL + F],
                        rhs=ind[:], start=(b == 0), stop=(b == nb - 1))
                return acc_ps

            # ---------------- phase 1
            with tc.tile_pool(name="g1", bufs=2) as gpool, \
                 tc.tile_pool(name="s1", bufs=2) as spool, \
                 tc.tile_pool(name="w1", bufs=3) as wpool, \
                 tc.tile_pool(name="pa1", bufs=2, space="PSUM") as ps_a, \
                 tc.tile_pool(name="pz1", bufs=2, space="PSUM") as ps_z, \
                 tc.tile_pool(name="pt1", bufs=2, space="PSUM") as ps_t:
                for g in range(GN):
                    xe, did_g, inv_g = gather_group(g, x_full, gpool, spool)
                    acc_ps = seg_sum(NB[g], xe, did_g, inv_g, IN_CH, ps_a,
                                     wpool, "1")
                    for tau in range(GT):
                        t = g * GT + tau
                        nc.vector.tensor_copy(
                            stacked1[:IN_CH, t * P:(t + 1) * P],
                            acc_ps[:, tau * P:(tau + 1) * P])
                        z_ps = ps_z.tile([P, HID], f32, tag="z")
                        nc.tensor.matmul(
                            z_ps[:], lhsT=stacked1[:, t * P:(t + 1) * P],
                            rhs=w1big_sb[:], start=True, stop=True)
                        h1t = wpool.tile([P, HID], bf16, tag="h1t")
                        nc.scalar.activation(
                            h1t[:], z_ps[:], mybir.ActivationFunctionType.Relu)
                        nc.sync.dma_start(h1_shard[t * P:(t + 1) * P, :],
                                          h1t[:])
                        h1T_ps = ps_t.tile([HID, P], bf16, tag="h1T")
                        nc.tensor.transpose(h1T_ps[:], h1t[:], identb_sb[:])
                        nc.vector.tensor_copy(
                            stacked2[HID:, t * P:(t + 1) * P], h1T_ps[:])

            nc.sync.dma_start(h1_shard[S - 1:S, :], zrow[:])
            if no_cc:
                nc.gpsimd.dma_start(h1_cat[:S, :], h1_shard[:])
            else:
                nc.gpsimd.collective_compute(
                    "AllGather", mybir.AluOpType.bypass, replica_groups=rg,
                    ins=[h1_shard[:].opt()], outs=[h1_cat[:].opt()])
            for w in range(n_cores):
                nc.sync.dma_start(h1_full[w * S:(w + 1) * S, :CC],
                                  h1_cat[w * S:(w + 1) * S, :])

            # ---------------- phase 2 + pooling
            with tc.tile_pool(name="g2", bufs=2) as gpool, \
                 tc.tile_pool(name="s2", bufs=2) as spool, \
                 tc.tile_pool(name="w2", bufs=3) as wpool, \
                 tc.tile_pool(name="pa2", bufs=2, space="PSUM") as ps_a, \
                 tc.tile_pool(name="pz2", bufs=2, space="PSUM") as ps_z, \
                 tc.tile_pool(name="pp2", bufs=2, space="PSUM") as ps_p:
                for g in range(GN):
                    xe, did_g, inv_g = gather_group(g, h1_full, gpool, spool)
                    acc_ps = seg_sum(NB[g], xe, did_g, inv_g, HID, ps_a,
                                     wpool, "2")
                    for tau in range(GT):
                        t = g * GT + tau
                        nc.vector.tensor_copy(
                            stacked2[:HID, t * P:(t + 1) * P],
                            acc_ps[:, tau * P:(tau + 1) * P])
                        z_ps = ps_z.tile([P, HID], f32, tag="z2")
                        nc.tensor.matmul(
                            z_ps[:], lhsT=stacked2[:, t * P:(t + 1) * P],
                            rhs=w2big_sb[:], start=True, stop=True)
                        zb = wpool.tile([P, HID], f32, tag="zb2")
                        nc.vector.tensor_tensor(zb[:], z_ps[:], b2rep_sb[:],
                                                op=mybir.AluOpType.add)
                        h2t = wpool.tile([P, HID], bf16, tag="h2t")
                        nc.scalar.activation(
                            h2t[:], zb[:], mybir.ActivationFunctionType.Relu)
                        if no_pool:
                            continue
                        indp = wpool.tile([P, POOL_SLOTS], bf16, tag="indp")
                        nc.vector.tensor_scalar(
                            indp[:], iota_sb[:], localgf_sb[:, t:t + 1], None,
                            op0=mybir.AluOpType.is_equal)
                        half = POOL_SLOTS // 2
                        for ch in range(2):
                            pp = ps_p.tile([HID, half], f32, tag="pp")
                            nc.tensor.matmul(
                                pp[:], lhsT=h2t[:],
                                rhs=indp[:, ch * half:(ch + 1) * half],
                                start=True, stop=True)
                            a = accT_sb[:, ch * half:(ch + 1) * half]
                            nc.vector.tensor_tensor(a, a, pp[:],
                                                    op=mybir.AluOpType.add)

            nc.sync.dma_start(pool[:], accT_sb[:])

    nc.compile()
    return nc


# ------------------------------------------------------------- kernel builder
def _build_fused(T, K, offs, C, S, n_cores, p1_k1=False, p2_k1=False,
                 no_pool=False, no_cc=False, nq=1):
    AS = n_cores * S
    nc = bacc.Bacc("TRN2", target_bir_lowering=False, debug=False,
                   enable_asserts=False, num_devices=n_cores,
                   num_swdge_queues=nq)
    qi = [0]

    def _q(inst):
        if nq > 1:
            qi[0] = (qi[0] + 1) % nq
            inst.ins.queue = f"qPoolDynamic{qi[0] or ''}"
        return inst
    xs = nc.dram_tensor("xs", [S, IN_CH], bf16, kind="ExternalInput").ap()
    nbr = nc.dram_tensor("nbr", [P, C], i32, kind="ExternalInput").ap()
    selfT1 = nc.dram_tensor("selfT1", [IN_CH, S], bf16,
                            kind="ExternalInput").ap()
    invdeg = nc.dram_tensor("invdeg", [P, T], f32, kind="ExternalInput").ap()
    maskf = nc.dram_tensor("maskf", [P, T], f32, kind="ExternalInput").ap()
    localgf = nc.dram_tensor("localgf", [P, T], f32, kind="ExternalInput").ap()
    iota = nc.dram_tensor("iota", [P, POOL_SLOTS], f32,
                          kind="ExternalInput").ap()
    identf = nc.dram_tensor("identf", [P, P], f32, kind="ExternalInput").ap()
    identb = nc.dram_tensor("identb", [P, P], bf16, kind="ExternalInput").ap()
    w1lT = nc.dram_tensor("w1lT", [IN_CH, HID], f32, kind="ExternalInput").ap()
    w1rT = nc.dram_tensor("w1rT", [IN_CH, HID], bf16,
                          kind="ExternalInput").ap()
    b1rep = nc.dram_tensor("b1rep", [P, HID], f32, kind="ExternalInput").ap()
    w2lT = nc.dram_tensor("w2lT", [HID, HID], f32, kind="ExternalInput").ap()
    w2rT = nc.dram_tensor("w2rT", [HID, HID], bf16, kind="ExternalInput").ap()
    b2rep = nc.dram_tensor("b2rep", [P, HID], f32, kind="ExternalInput").ap()
    pool = nc.dram_tensor("pool", [POOL_SLOTS, HID], f32,
                          kind="ExternalOutput").ap()

    rg = [list(range(n_cores))]
    with tile.TileContext(nc) as tc:
        with tc.tile_pool(name="dramp", bufs=1, space="DRAM") as dpool, \
             tc.tile_pool(name="resident", bufs=1) as rpool:
            x_shard = dpool.tile([S, IN_CH], bf16, tag="x_shard")
            x_full = dpool.tile([AS, IN_CH], bf16, addr_space="Shared",
                                tag="x_full")
            h1_shard = dpool.tile([S, HID], bf16, tag="h1_shard")
            h1_full = dpool.tile([AS, HID], bf16, addr_space="Shared",
                                 tag="h1_full")

            # phase 0: AllGather x shards into the full (permuted) table
            nc.gpsimd.dma_start(x_shard[:], xs[:])
            if no_cc:
                nc.gpsimd.dma_start(x_full[:S, :], x_shard[:])
            else:
                nc.gpsimd.collective_compute(
                    "AllGather", mybir.AluOpType.bypass, replica_groups=rg,
                    ins=[x_shard[:].opt()], outs=[x_full[:].opt()])

            nbr_sb = rpool.tile([P, C], i32, tag="nbr")
            nc.sync.dma_start(nbr_sb[:], nbr[:])
            selfT1_sb = rpool.tile([IN_CH, S], bf16, tag="selfT1")
            nc.sync.dma_start(selfT1_sb[:], selfT1[:])
            invdeg_sb = rpool.tile([P, T], f32, tag="invdeg")
            nc.sync.dma_start(invdeg_sb[:], invdeg[:])
            maskf_sb = rpool.tile([P, T], f32, tag="maskf")
            nc.sync.dma_start(maskf_sb[:], maskf[:])
            localgf_sb = rpool.tile([P, T], f32, tag="localgf")
            nc.sync.dma_start(localgf_sb[:], localgf[:])
            iota_sb = rpool.tile([P, POOL_SLOTS], f32, tag="iota")
            nc.sync.dma_start(iota_sb[:], iota[:])
            identf_sb = rpool.tile([P, P], f32, tag="identf")
            nc.sync.dma_start(identf_sb[:], identf[:])
            identb_sb = rpool.tile([P, P], bf16, tag="identb")
            nc.sync.dma_start(identb_sb[:], identb[:])
            w1lT_sb = rpool.tile([IN_CH, HID], f32, tag="w1lT")
            nc.sync.dma_start(w1lT_sb[:], w1lT[:])
            w1rT_sb = rpool.tile([IN_CH, HID], bf16, tag="w1rT")
            nc.sync.dma_start(w1rT_sb[:], w1rT[:])
            b1rep_sb = rpool.tile([P, HID], f32, tag="b1rep")
            nc.sync.dma_start(b1rep_sb[:], b1rep[:])
            w2lT_sb = rpool.tile([HID, HID], f32, tag="w2lT")
            nc.sync.dma_start(w2lT_sb[:], w2lT[:])
            w2rT_sb = rpool.tile([HID, HID], bf16, tag="w2rT")
            nc.sync.dma_start(w2rT_sb[:], w2rT[:])
            b2rep_sb = rpool.tile([P, HID], f32, tag="b2rep")
            nc.sync.dma_start(b2rep_sb[:], b2rep[:])
            selfT2_sb = rpool.tile([HID, S], bf16, tag="selfT2")
            acc_sb = rpool.tile([P, POOL_CHUNKS * HID], f32, tag="acc")
            nc.vector.memset(acc_sb[:], 0.0)

            # ---------------- phase 1: layer 1 over x_full
            with tc.tile_pool(name="gather1", bufs=3) as gpool, \
                 tc.tile_pool(name="work1", bufs=3) as wpool, \
                 tc.tile_pool(name="ps_t1", bufs=2, space="PSUM") as ps_t, \
                 tc.tile_pool(name="ps_h1", bufs=2, space="PSUM") as ps_h, \
                 tc.tile_pool(name="ps_z1", bufs=2, space="PSUM") as ps_z:
                for t in range(T):
                    kt = 1 if p1_k1 else K[t]
                    g = gpool.tile([P, kt * IN_CH], bf16, tag="g")
                    for k in range(kt):
                        _q(nc.gpsimd.indirect_dma_start(
                            out=g[:, k * IN_CH:(k + 1) * IN_CH],
                            out_offset=None, in_=x_full[:],
                            in_offset=bass.IndirectOffsetOnAxis(
                                ap=nbr_sb[:, offs[t] + k:offs[t] + k + 1],
                                axis=0)))
                    agg = wpool.tile([P, IN_CH], f32, tag="agg")
                    nc.vector.tensor_reduce(
                        agg[:], g[:].rearrange("p (k f) -> p f k", k=kt),
                        axis=mybir.AxisListType.X, op=mybir.AluOpType.add)
                    nc.vector.tensor_scalar_mul(agg[:], agg[:],
                                                invdeg_sb[:, t:t + 1])
                    aggT_ps = ps_t.tile([IN_CH, P], f32, tag="aggT")
                    nc.tensor.transpose(aggT_ps[:], agg[:], identf_sb[:])
                    aggT = wpool.tile([IN_CH, P], f32, tag="aggTs")
                    nc.vector.tensor_copy(aggT[:], aggT_ps[:])
                    z_ps = ps_z.tile([P, HID], f32, tag="z")
                    nc.tensor.matmul(z_ps[:], lhsT=aggT[:], rhs=w1lT_sb[:],
                                     start=True, stop=False)
                    nc.tensor.matmul(z_ps[:],
                                     lhsT=selfT1_sb[:, t * P:(t + 1) * P],
                                     rhs=w1rT_sb[:], start=False, stop=True)
                    zb = wpool.tile([P, HID], f32, tag="zb")
                    nc.vector.tensor_tensor(zb[:], z_ps[:], b1rep_sb[:],
                                            op=mybir.AluOpType.add)
                    nc.vector.tensor_scalar_mul(zb[:], zb[:],
                                                maskf_sb[:, t:t + 1])
                    h1t = wpool.tile([P, HID], bf16, tag="h1t")
                    nc.scalar.activation(h1t[:], zb[:],
                                         mybir.ActivationFunctionType.Relu)
                    nc.sync.dma_start(h1_shard[t * P:(t + 1) * P, :], h1t[:])
                    h1T_ps = ps_h.tile([HID, P], bf16, tag="h1T")
                    nc.tensor.transpose(h1T_ps[:], h1t[:], identb_sb[:])
                    nc.vector.tensor_copy(selfT2_sb[:, t * P:(t + 1) * P],
                                          h1T_ps[:])

            # AllGather h1 across cores
            if no_cc:
                nc.gpsimd.dma_start(h1_full[:S, :], h1_shard[:])
            else:
                nc.gpsimd.collective_compute(
                    "AllGather", mybir.AluOpType.bypass, replica_groups=rg,
                    ins=[h1_shard[:].opt()], outs=[h1_full[:].opt()])

            # ---------------- phase 2: layer 2 over h1_full + pooling
            with tc.tile_pool(name="gather2", bufs=3) as gpool, \
                 tc.tile_pool(name="work2", bufs=3) as wpool, \
                 tc.tile_pool(name="ps_t2", bufs=2, space="PSUM") as ps_t, \
                 tc.tile_pool(name="ps_z2", bufs=2, space="PSUM") as ps_z, \
                 tc.tile_pool(name="ps_p2", bufs=2, space="PSUM") as ps_p:
                for t in range(T):
                    kt = 1 if p2_k1 else K[t]
                    g = gpool.tile([P, kt * HID], bf16, tag="g2")
                    for k in range(kt):
                        _q(nc.gpsimd.indirect_dma_start(
                            out=g[:, k * HID:(k + 1) * HID],
                            out_offset=None, in_=h1_full[:],
                            in_offset=bass.IndirectOffsetOnAxis(
                                ap=nbr_sb[:, offs[t] + k:offs[t] + k + 1],
                                axis=0)))
                    agg = wpool.tile([P, HID], f32, tag="agg2")
                    nc.vector.tensor_reduce(
                        agg[:], g[:].rearrange("p (k f) -> p f k", k=kt),
                        axis=mybir.AxisListType.X, op=mybir.AluOpType.add)
                    nc.vector.tensor_scalar_mul(agg[:], agg[:],
                                                invdeg_sb[:, t:t + 1])
                    aggT_ps = ps_t.tile([HID, P], f32, tag="aggT2")
                    nc.tensor.transpose(aggT_ps[:], agg[:], identf_sb[:])
                    aggT = wpool.tile([HID, P], f32, tag="aggTs2")
                    nc.vector.tensor_copy(aggT[:], aggT_ps[:])
                    z_ps = ps_z.tile([P, HID], f32, tag="z2")
                    nc.tensor.matmul(z_ps[:], lhsT=aggT[:], rhs=w2lT_sb[:],
                                     start=True, stop=False)
                    nc.tensor.matmul(z_ps[:],
                                     lhsT=selfT2_sb[:, t * P:(t + 1) * P],
                                     rhs=w2rT_sb[:], start=False, stop=True)
                    zb = wpool.tile([P, HID], f32, tag="zb2")
                    nc.vector.tensor_tensor(zb[:], z_ps[:], b2rep_sb[:],
                                            op=mybir.AluOpType.add)
                    h2t = wpool.tile([P, HID], bf16, tag="h2t")
                    nc.scalar.activation(h2t[:], zb[:],
                                         mybir.ActivationFunctionType.Relu)
                    if no_pool:
                        continue
                    # one-hot pooling: ind[p, s] = (localg[p] == s)
                    ind = wpool.tile([P, POOL_SLOTS], bf16, tag="ind")
                    nc.vector.tensor_scalar(
                        ind[:], iota_sb[:], localgf_sb[:, t:t + 1], None,
                        op0=mybir.AluOpType.is_equal)
                    for ch in range(POOL_CHUNKS):
                        pp = ps_p.tile([P, HID], f32, tag="pp")
                        nc.tensor.matmul(
                            pp[:], lhsT=ind[:, ch * P:(ch + 1) * P],
                            rhs=h2t[:], start=True, stop=True)
                        a = acc_sb[:, ch * HID:(ch + 1) * HID]
                        nc.vector.tensor_tensor(a, a, pp[:],
                                                op=mybir.AluOpType.add)

            for ch in range(POOL_CHUNKS):
                nc.sync.dma_start(pool[ch * P:(ch + 1) * P, :],
                                  acc_sb[:, ch * HID:(ch + 1) * HID])

    nc.compile()
    return nc


# ---------------------------------------------------------------- jit runner
class _PjrtRunner:
    """Persistent jit(shard_map) executor for a compiled Bass module.

    Same _bass_exec_p machinery run_bass_kernel_spmd uses under axon, but
    the jitted callable and device-resident inputs persist across calls so
    steady-state executions can be timed without re-trace/re-compile or
    host->device traffic.
    """

    def __init__(self, nc, n_cores):
        import jax
        from jax.experimental.shard_map import shard_map
        from jax.sharding import Mesh, NamedSharding, PartitionSpec
        from concourse import bass2jax

        bass2jax.install_neuronx_cc_hook()
        self.jax = jax
        self.nc = nc
        self.n_cores = n_cores
        partition_name = (nc.partition_id_tensor.name
                          if nc.partition_id_tensor else None)
        in_names, out_names, out_avals, zero_shapes = [], [], [], []
        for alloc in nc.m.functions[0].allocations:
            if not isinstance(alloc, mybir.MemoryLocationSet):
                continue
            name = alloc.memorylocations[0].name
            if alloc.kind == "ExternalInput":
                if name != partition_name:
                    in_names.append(name)
            elif alloc.kind == "ExternalOutput":
                shape = tuple(alloc.tensor_shape)
                dtype = mybir.dt.np(alloc.dtype)
                out_names.append(name)
                out_avals.append(jax.core.ShapedArray(shape, dtype))
                zero_shapes.append((shape, dtype))
        self.in_names = list(in_names)
        self.out_names = out_names
        self.zero_shapes = zero_shapes
        n_params = len(in_names)
        n_outs = len(out_names)
        all_names = in_names + out_names
        if partition_name is not None:
            all_names.append(partition_name)
        donate = tuple(range(n_params, n_params + n_outs))

        def _body(*args):
            operands = list(args)
            if partition_name is not None:
                operands.append(bass2jax.partition_id_tensor())
            outs = bass2jax._bass_exec_p.bind(
                *operands,
                out_avals=tuple(out_avals),
                in_names=tuple(all_names),
                out_names=tuple(out_names),
                lowering_input_output_aliases=(),
                sim_require_finite=True,
                sim_require_nnan=True,
                nc=nc,
            )
            return tuple(outs)

        devices = jax.devices()[:n_cores]
        assert len(devices) == n_cores
        self.mesh = Mesh(np.asarray(devices), ("core",))
        self.sharding = NamedSharding(self.mesh, PartitionSpec("core"))
        in_specs = (PartitionSpec("core"),) * (n_params + n_outs)
        out_specs = (PartitionSpec("core"),) * n_outs
        self._fn = jax.jit(
            shard_map(_body, mesh=self.mesh, in_specs=in_specs,
                      out_specs=out_specs, check_rep=False),
            donate_argnums=donate, keep_unused=True)
        self._dev_inputs = None

    def put_inputs(self, in_maps):
        """Concatenate per-core inputs on axis 0 and place on device."""
        jax = self.jax
        self._dev_inputs = []
        for name in self.in_names:
            arr = np.concatenate([np.asarray(m[name]) for m in in_maps],
                                 axis=0)
            self._dev_inputs.append(
                jax.device_put(arr, self.sharding))
        jax.block_until_ready(self._dev_inputs)

    def _zeros(self):
        jax = self.jax
        zs = [jax.device_put(
                  np.zeros((self.n_cores * s[0], *s[1:]), d), self.sharding)
              for s, d in self.zero_shapes]
        jax.block_until_ready(zs)
        return zs

    def run(self, zeros=None):
        jax = self.jax
        if zeros is None:
            zeros = self._zeros()
        outs = self._fn(*self._dev_inputs, *zeros)
        jax.block_until_ready(outs)
        return outs

    def results(self, outs):
        per_core = []
        for c in range(self.n_cores):
            m = {}
            for i, name in enumerate(self.out_names):
                shape, _ = self.zero_shapes[i]
                m[name] = np.asarray(outs[i]).reshape(
                    self.n_cores, *shape)[c]
            per_core.append(m)
        return per_core

    def time_runs(self, reps=3):
        zero_sets = [self._zeros() for _ in range(reps)]
        times = []
        for zs in zero_sets:
            t0 = time.perf_counter()
            outs = self._fn(*self._dev_inputs, *zs)
            self.jax.block_until_ready(outs)
            times.append(time.perf_counter() - t0)
        return times

    def time_pipelined(self, reps=10):
        """Issue `reps` executions back-to-back (async dispatch), block once.

        Per-run time = total / reps: dispatch latency overlaps execution, so
        this converges to the device execution+collective time per run.
        """
        zero_sets = [self._zeros() for _ in range(reps)]
        all_outs = []
        t0 = time.perf_counter()
        for zs in zero_sets:
            all_outs.append(self._fn(*self._dev_inputs, *zs))
        self.jax.block_until_ready(all_outs)
        return (time.perf_counter() - t0) / reps


# -------------------------------------------------------------------- kernel
def _kernel_impl(x, edge_index, batch, W1_l, b1, W1_r, W2_l, b2, W2_r,
                 W_lin, b_lin, n_nodes, n_graphs, n_cores, _timing=None,
                 mode="eg"):
    x = np.asarray(x, dtype=np.float32)
    W1_l = np.asarray(W1_l, np.float32)
    W1_r = np.asarray(W1_r, np.float32)
    W2_l = np.asarray(W2_l, np.float32)
    W2_r = np.asarray(W2_r, np.float32)
    b1 = np.asarray(b1, np.float32)
    b2 = np.asarray(b2, np.float32)
    W_lin = np.asarray(W_lin, np.float32)
    b_lin = np.asarray(b_lin, np.float32)

    iota = np.broadcast_to(
        np.arange(POOL_SLOTS, dtype=np.float32), (P, POOL_SLOTS)).copy()
    identb = np.eye(P).astype(bfl)
    b2rep = np.broadcast_to(b2, (P, HID)).copy()

    t0 = time.time()
    if mode == "eg":
        pp = _prep_eg(edge_index, batch, n_nodes, n_graphs, n_cores)
        T, S = pp["T"], pp["S"]
        t_prep = time.time() - t0

        t0 = time.time()
        nc = _build_eg(T, S, pp["GN"], pp["NB"], pp["POS"], pp["pos_base"],
                       pp["segs"], n_cores)
        t_build = time.time() - t0

        w1big = np.zeros((2 * IN_CH + 1, HID), np.float32)
        w1big[:IN_CH] = W1_l.T
        w1big[IN_CH:2 * IN_CH] = W1_r.T
        w1big[2 * IN_CH] = b1
        w2big = np.concatenate([W2_l.T, W2_r.T], axis=0)
        in_maps = []
        for ci in pp["cores"]:
            xs = np.zeros((S, 64), bfl)
            xs[:ci["n"], :IN_CH] = x[ci["ids"]]
            in_maps.append(dict(
                xs=xs,
                selfT1=np.ascontiguousarray(xs[:, :IN_CH].T),
                idx=ci["idx"], dinv=ci["dinv"],
                localgf=ci["localgf"], iota=iota, identb=identb,
                w1big=w1big.astype(bfl), w2big=w2big.astype(bfl),
                b2rep=b2rep))
    else:
        pp = _prep(edge_index, batch, n_nodes, n_graphs, n_cores)
        T, S, K, offs, C = pp["T"], pp["S"], pp["K"], pp["offs"], pp["C"]
        t_prep = time.time() - t0

        t0 = time.time()
        nc = _build_fused(T, K, offs, C, S, n_cores)
        t_build = time.time() - t0

        identf = np.eye(P, dtype=np.float32)
        b1rep = np.broadcast_to(b1, (P, HID)).copy()
        in_maps = []
        for ci in pp["cores"]:
            xs = np.zeros((S, IN_CH), bfl)
            xs[:ci["n"]] = x[ci["ids"]]
            in_maps.append(dict(
                xs=xs, nbr=ci["nbr"],
                selfT1=np.ascontiguousarray(xs.T),
                invdeg=ci["invdeg"], maskf=ci["maskf"], localgf=ci["localgf"],
                iota=iota, identf=identf, identb=identb,
                w1lT=np.ascontiguousarray(W1_l.T),
                w1rT=np.ascontiguousarray(W1_r.T).astype(bfl),
                b1rep=b1rep,
                w2lT=np.ascontiguousarray(W2_l.T),
                w2rT=np.ascontiguousarray(W2_r.T).astype(bfl),
                b2rep=b2rep))

    t0 = time.time()
    runner = _PjrtRunner(nc, n_cores)
    runner.put_inputs(in_maps)
    t_put = time.time() - t0

    t0 = time.time()
    outs = runner.run()  # first call compiles (NEFF via neuronx_cc hook)
    t_first = time.time() - t0
    res = runner.results(outs)

    # host: divide pooled sums by graph node counts; final linear
    pool = np.zeros((n_graphs, HID), np.float32)
    for c, (ci, r) in enumerate(zip(pp["cores"], res)):
        ng = ci["ghi"] - ci["glo"]
        rp = r["pool"]
        if rp.shape[0] == HID:  # eg kernel outputs [HID, POOL_SLOTS]
            rp = np.ascontiguousarray(rp.T)
        pool[ci["glo"]:ci["ghi"]] = rp[:ng]
    gcnt = np.maximum(pp["gcnt"], 1).astype(np.float32)
    pooled = pool / gcnt[:, None]
    out = (pooled @ W_lin.T + b_lin).astype(np.float32)

    if _timing is not None:
        times = runner.time_runs(reps=2)
        tot20 = runner.time_pipelined(reps=20) * 20
        tot40 = runner.time_pipelined(reps=40) * 40
        _timing.update(prep=t_prep, build=t_build, put=t_put,
                       first=t_first, reps=times,
                       piped40=tot40 / 40, marginal=(tot40 - tot20) / 20,
                       exec_ns=tot40 / 40 * 1e9)
    return out


def kernel(x, edge_index, batch, W1_l, b1, W1_r, W2_l, b2, W2_r, W_lin,
           b_lin, _timing=None):
    return _kernel_impl(x, edge_index, batch, W1_l, b1, W1_r, W2_l, b2,
                        W2_r, W_lin, b_lin, N_NODES, N_GRAPHS, N_CORES,
                        _timing=_timing)


# revision 31
# speedup vs baseline: 1.0379x; 1.0379x over previous
"""Trainium2 fused Bass kernel for JetGNN (2-layer SAGEConv + global mean pool).

Single fused NEFF, SPMD x8, graph-aligned node sharding (batch is sorted, so
whole jets stay on one core and pooling never crosses cores). Message passing
is edge-major around the SWDGE dma_gather primitive:

  - Host: per-core nodes are degree-sorted; edges are bucketed per
    (4-tile dst group, source-core window), sorted, padded to a core-uniform
    layout; window-local int16 gather indices + per-edge (dst-slot, 1/deg)
    tables; pad edges point at a guaranteed-zero row.
  - Device:  AllGather x shards (bf16, compact [S,64]) -> spread to 256B-pitch
    rows (dma_gather granularity).  Per group: dma_gather the edges' source
    rows (4 SWDGE queues round-robin), build one-hot indicators
    (is_equal vs iota, scaled by 1/deg via the fused second scalar op) and
    segment-sum via PSUM-accumulated matmuls -> mean^T directly (no PE
    transpose).  The layer matmul stacks [mean; self; ones] so W_l, W_r and
    the bias fold into one PE op.  h1 -> DRAM, AllGather, spread, repeat for
    layer 2; global pooling via two transposed one-hot matmuls per tile into
    a resident SBUF accumulator.
  - Host: divide pooled sums by graph node counts, final 64->2 linear.

Timing: a persistent jit(shard_map) runner (the same _bass_exec_p machinery
run_bass_kernel_spmd uses under axon) compiles once, keeps inputs on device,
and reports steady-state per-run time from 40 pipelined executions
(block_until_ready once at the end; the axon tunnel adds ~70ms dispatch
latency per blocking call, which pipelining amortizes away).
"""

import math
import time

import numpy as np
import ml_dtypes

import concourse.bass as bass
import concourse.tile as tile
import concourse.mybir as mybir
from concourse import bacc

N_NODES = 200000
N_GRAPHS = 4000
N_CORES = 8
IN_CH = 32
HID = 64
P = 128
POOL_SLOTS = 640  # per-core graph slots (~500 graphs/core); last slot = dump
DUMP_SLOT = POOL_SLOTS - 1
POOL_CHUNKS = POOL_SLOTS // P

f32 = mybir.dt.float32
bf16 = mybir.dt.bfloat16
i32 = mybir.dt.int32
bfl = ml_dtypes.bfloat16


# ----------------------------------------------------------------- host prep
def _prep(edge_index, batch, n_nodes, n_graphs, n_cores):
    """Vectorized host prep. Returns layout + per-core tables."""
    src = np.asarray(edge_index[0]).astype(np.int64)
    dst = np.asarray(edge_index[1]).astype(np.int64)
    batch = np.asarray(batch).astype(np.int64)
    deg = np.bincount(dst, minlength=n_nodes).astype(np.int64)

    # CSR by dst
    order = np.argsort(dst, kind="stable")
    src_sorted = src[order]
    rowptr = np.zeros(n_nodes + 1, dtype=np.int64)
    np.cumsum(deg, out=rowptr[1:])

    # graph-aligned core boundaries (batch is sorted by graph id)
    gcnt = np.bincount(batch, minlength=n_graphs)
    gends = np.cumsum(gcnt)
    targets = (np.arange(1, n_cores) * n_nodes) // n_cores
    gb = np.searchsorted(gends, targets)
    graph_bounds = np.concatenate([[0], gb + 1, [n_graphs]])
    node_bounds = np.concatenate([[0], gends[graph_bounds[1:-1] - 1], [n_nodes]])

    cores = []
    for c in range(n_cores):
        lo, hi = int(node_bounds[c]), int(node_bounds[c + 1])
        perm = np.argsort(-deg[lo:hi], kind="stable")
        ids = np.arange(lo, hi)[perm]
        glo, ghi = int(graph_bounds[c]), int(graph_bounds[c + 1])
        assert ghi - glo <= DUMP_SLOT
        cores.append(dict(ids=ids, glo=glo, ghi=ghi, n=hi - lo))

    T = max(1, math.ceil(max(ci["n"] for ci in cores) / P))
    S = T * P
    # per-tile K: max over cores of max degree within the tile
    K = np.ones(T, dtype=np.int64)
    for ci in cores:
        d = np.pad(deg[ci["ids"]], (0, S - ci["n"]))
        K = np.maximum(K, d.reshape(T, P).max(axis=1))
    offs = np.zeros(T + 1, dtype=np.int64)
    np.cumsum(K, out=offs[1:])
    C = int(offs[-1])

    # global position of each node in the AllGathered [n_cores*S] layout
    global_pos = np.empty(n_nodes, dtype=np.int64)
    for c, ci in enumerate(cores):
        global_pos[ci["ids"]] = c * S + np.arange(ci["n"])
    pad_core = next(c for c, ci in enumerate(cores) if ci["n"] < S)
    pad_slot = pad_core * S + cores[pad_core]["n"]  # guaranteed all-zero row

    for c, ci in enumerate(cores):
        ids, n = ci["ids"], ci["n"]
        deg_c = deg[ids]
        tot = int(deg_c.sum())
        # edge-wise coordinates: node at position pos -> tile t, row p
        cum0 = np.zeros(n, dtype=np.int64)
        np.cumsum(deg_c[:-1], out=cum0[1:])
        e_node = np.repeat(np.arange(n), deg_c)  # position of dst node
        j = np.arange(tot) - np.repeat(cum0, deg_c)  # slot within nbr list
        e_idx = np.repeat(rowptr[ids], deg_c) + j  # CSR edge index
        t_e = e_node // P
        p_e = e_node % P
        col = offs[t_e] + j
        nbr = np.full(P * C, pad_slot, dtype=np.int32)
        nbr[p_e * C + col] = global_pos[src_sorted[e_idx]].astype(np.int32)

        def _tileize(vals, pad_val, dtype):
            a = np.full(S, pad_val, dtype=dtype)
            a[:n] = vals
            return np.ascontiguousarray(a.reshape(T, P).T)

        invdeg = _tileize(1.0 / np.maximum(deg_c, 1), 0.0, np.float32)
        maskf = _tileize(np.ones(n), 0.0, np.float32)
        localgf = _tileize(batch[ids] - ci["glo"], DUMP_SLOT, np.float32)
        ci.update(nbr=nbr.reshape(P, C), invdeg=invdeg, maskf=maskf,
                  localgf=localgf)

    return dict(cores=cores, T=T, S=S, K=K.tolist(), offs=offs.tolist(), C=C,
                node_bounds=node_bounds, graph_bounds=graph_bounds,
                gcnt=gcnt, batch=batch)


# ----------------------------------------------------- edge-major host prep
GT = 4  # dst tiles per group (512 slots -> one 2KB PSUM bank at f32)
GATHER_CHUNK = 512  # idxs per dma_gather (best 4-queue balance measured)
SENT = 4096.0  # dstid sentinel for pad edges (matches no iota column)


def _prep_eg(edge_index, batch, n_nodes, n_graphs, n_cores):
    """Edge-major prep for the dma_gather kernel.

    Edges are bucketed per (dst-tile-group, source-core window), sorted, and
    padded to a single structure shared by all cores (max over cores), so the
    SPMD program is uniform. Indices are window-local (int16) positions into
    the AllGathered [n_cores*S] node table.
    """
    src = np.asarray(edge_index[0]).astype(np.int64)
    dst = np.asarray(edge_index[1]).astype(np.int64)
    batch = np.asarray(batch).astype(np.int64)
    deg = np.bincount(dst, minlength=n_nodes).astype(np.int64)

    gcnt = np.bincount(batch, minlength=n_graphs)
    gends = np.cumsum(gcnt)
    targets = (np.arange(1, n_cores) * n_nodes) // n_cores
    gb = np.searchsorted(gends, targets)
    graph_bounds = np.concatenate([[0], gb + 1, [n_graphs]])
    node_bounds = np.concatenate([[0], gends[graph_bounds[1:-1] - 1],
                                  [n_nodes]])

    cores = []
    for c in range(n_cores):
        lo, hi = int(node_bounds[c]), int(node_bounds[c + 1])
        perm = np.argsort(-deg[lo:hi], kind="stable")
        ids = np.arange(lo, hi)[perm]
        glo, ghi = int(graph_bounds[c]), int(graph_bounds[c + 1])
        assert ghi - glo <= DUMP_SLOT
        slot = np.empty(hi - lo, dtype=np.int64)
        slot[perm] = np.arange(hi - lo)
        cores.append(dict(ids=ids, glo=glo, ghi=ghi, n=hi - lo, lo=lo, hi=hi,
                          slot=slot))

    max_n = max(ci["n"] for ci in cores)
    T = math.ceil((max_n + 1) / P)  # +1: every core keeps a zero pad row
    T = ((T + GT - 1) // GT) * GT
    S = T * P
    GN = T // GT
    GS = GT * P  # slots per group

    global_pos = np.empty(n_nodes, dtype=np.int64)
    for c, ci in enumerate(cores):
        global_pos[ci["ids"]] = c * S + np.arange(ci["n"])

    # per-core edge buckets -> uniform (max over cores) segment sizes
    cnts = np.zeros((n_cores, GN, n_cores), dtype=np.int64)
    percore = []
    for c, ci in enumerate(cores):
        emask = (dst >= ci["lo"]) & (dst < ci["hi"])
        ed, es = dst[emask], src[emask]
        dslot = ci["slot"][ed - ci["lo"]]
        sg = global_pos[es]
        g_e = dslot // GS
        w_e = sg // S
        order = np.lexsort((dslot, w_e, g_e))
        percore.append(dict(
            g=g_e[order], w=w_e[order],
            loc=(sg[order] % S).astype(np.int16),
            did=(dslot[order] - g_e[order] * GS).astype(np.float32),
            inv=(1.0 / deg[ed[order]]).astype(np.float32)))
        np.add.at(cnts[c], (g_e, w_e), 1)

    Lgw = ((cnts.max(axis=0) + P - 1) // P) * P  # [GN, n_cores]
    # guarantee >= 1 block per group (isolated/pad-only groups)
    empty_g = Lgw.sum(axis=1) == 0
    Lgw[empty_g, 0] = P
    NB = (Lgw.sum(axis=1) // P).astype(np.int64)  # blocks per group
    POS = NB * P
    seg_base = np.zeros((GN, n_cores), dtype=np.int64)
    pos_base = np.zeros(GN + 1, dtype=np.int64)
    for g in range(GN):
        pos_base[g + 1] = pos_base[g] + POS[g]
        seg_base[g] = pos_base[g] + np.concatenate(
            [[0], np.cumsum(Lgw[g][:-1])])
    POSTOT = int(pos_base[-1])
    NBTOT = POSTOT // P
    # static segments for codegen: per group, (window, n_blocks, idx col/16,
    # out block offset) — chunked to <= 2048 idxs
    segs = []
    for g in range(GN):
        sg_list = []
        for w in range(n_cores):
            L = int(Lgw[g][w])
            off = int(seg_base[g][w] - pos_base[g])
            while L > 0:
                chunk = min(L, GATHER_CHUNK)
                sg_list.append((w, chunk // P, off // 16, off // P))
                off += chunk
                L -= chunk
        segs.append(sg_list)

    for c, ci in enumerate(cores):
        pc = percore[c]
        cell = pc["g"] * n_cores + pc["w"]
        cell_counts = np.bincount(cell, minlength=GN * n_cores)
        cell_start = np.concatenate([[0], np.cumsum(cell_counts)[:-1]])
        rank = np.arange(len(cell)) - cell_start[cell]
        tgt = seg_base.reshape(-1)[cell] + rank
        idxf = np.full(POSTOT, S - 1, dtype=np.int16)
        didf = np.full(POSTOT, SENT, dtype=np.float32)
        invf = np.zeros(POSTOT, dtype=np.float32)
        idxf[tgt] = pc["loc"]
        didf[tgt] = pc["did"]
        invf[tgt] = pc["inv"]
        ci["idx"] = np.tile(np.ascontiguousarray(
            idxf.reshape(-1, 16).T), (n_cores, 1))
        ci["dstid"] = np.ascontiguousarray(didf.reshape(-1, P).T)
        ci["invde"] = np.ascontiguousarray(invf.reshape(-1, P).T)
        nbtot = POSTOT // P
        dinv = np.empty((P, 2 * nbtot), np.float32)
        for g in range(GN):
            bb = pos_base[g] // P
            nb = int(NB[g])
            dinv[:, 2 * bb:2 * bb + nb] = ci["dstid"][:, bb:bb + nb]
            dinv[:, 2 * bb + nb:2 * (bb + nb)] = ci["invde"][:, bb:bb + nb]
        ci["dinv"] = dinv
        lg = np.full(S, DUMP_SLOT, dtype=np.float32)
        lg[:ci["n"]] = batch[ci["ids"]] - ci["glo"]
        ci["localgf"] = np.ascontiguousarray(lg.reshape(T, P).T)

    # in-group slice offsets for per-group table streams
    return dict(cores=cores, T=T, S=S, GN=GN, NB=NB.tolist(),
                POS=POS.tolist(), pos_base=pos_base.tolist(), segs=segs,
                gcnt=gcnt, batch=batch)


# ------------------------------------------------- edge-major kernel builder
def _build_eg(T, S, GN, NB, POS, pos_base, segs, n_cores, no_gather=False,
              with_lib=True, no_cc=False, no_pool=False, no_ind=False,
              no_bmm=False, scratch=None, nq=4, gbufs=2, wbufs=3):
    from concourse.library_config import mlp as mlp_lib

    AS = n_cores * S
    XCOL = 128  # gathered row width (256B at bf16, dma_gather granularity)
    CC = 64  # compact row width for DRAM tables / collectives
    NBTOT = pos_base[-1] // P
    kw = dict(dynamic_dma_scratch_size=scratch) if scratch else {}
    nc = bacc.Bacc("TRN2", target_bir_lowering=False, debug=False,
                   enable_asserts=False, num_devices=n_cores,
                   num_swdge_queues=nq, **kw)
    i16 = mybir.dt.int16
    qrr = [0]
    xs = nc.dram_tensor("xs", [S, CC], bf16, kind="ExternalInput").ap()
    selfT1 = nc.dram_tensor("selfT1", [IN_CH, S], bf16,
                            kind="ExternalInput").ap()
    idx = nc.dram_tensor("idx", [P, pos_base[-1] // 16], i16,
                         kind="ExternalInput").ap()
    dinv = nc.dram_tensor("dinv", [P, 2 * NBTOT], f32,
                          kind="ExternalInput").ap()
    localgf = nc.dram_tensor("localgf", [P, T], f32, kind="ExternalInput").ap()
    iota = nc.dram_tensor("iota", [P, POOL_SLOTS], f32,
                          kind="ExternalInput").ap()
    identb = nc.dram_tensor("identb", [P, P], bf16, kind="ExternalInput").ap()
    w1big = nc.dram_tensor("w1big", [2 * IN_CH + 1, HID], bf16,
                           kind="ExternalInput").ap()
    w2big = nc.dram_tensor("w2big", [2 * HID, HID], bf16,
                           kind="ExternalInput").ap()
    b2rep = nc.dram_tensor("b2rep", [P, HID], f32, kind="ExternalInput").ap()
    pool = nc.dram_tensor("pool", [HID, POOL_SLOTS], f32,
                          kind="ExternalOutput").ap()

    rg = [list(range(n_cores))]
    with tile.TileContext(nc) as tc:
        if with_lib:
            nc.gpsimd.load_library(mlp_lib)
        with tc.tile_pool(name="dramp", bufs=1, space="DRAM") as dpool, \
             tc.tile_pool(name="resident", bufs=1) as rpool:
            x_shard = dpool.tile([S, CC], bf16, tag="x_shard")
            x_cat = dpool.tile([AS, CC], bf16, addr_space="Shared",
                               tag="x_cat")
            x_full = dpool.tile([AS, XCOL], bf16, tag="x_full")
            h1_shard = dpool.tile([S, CC], bf16, tag="h1_shard")
            h1_cat = dpool.tile([AS, CC], bf16, addr_space="Shared",
                                tag="h1_cat")
            h1_full = dpool.tile([AS, XCOL], bf16, tag="h1_full")

            nc.gpsimd.dma_start(x_shard[:], xs[:])
            if no_cc:
                nc.gpsimd.dma_start(x_cat[:S, :], x_shard[:])
            else:
                nc.gpsimd.collective_compute(
                    "AllGather", mybir.AluOpType.bypass, replica_groups=rg,
                    ins=[x_shard[:].opt()], outs=[x_cat[:].opt()])
            # spread compact rows to 256B pitch for dma_gather
            # (split: DMA AP dim counts are 16-bit)
            for w in range(n_cores):
                nc.sync.dma_start(x_full[w * S:(w + 1) * S, :CC],
                                  x_cat[w * S:(w + 1) * S, :])

            stacked1 = rpool.tile([2 * IN_CH + 1, T * P], bf16,
                                  tag="stacked1")
            nc.sync.dma_start(stacked1[IN_CH:2 * IN_CH, :], selfT1[:])
            nc.vector.memset(stacked1[2 * IN_CH:2 * IN_CH + 1, :], 1.0)
            stacked2 = rpool.tile([2 * HID, T * P], bf16, tag="stacked2")
            localgf_sb = rpool.tile([P, T], f32, tag="localgf")
            nc.sync.dma_start(localgf_sb[:], localgf[:])
            iota_sb = rpool.tile([P, POOL_SLOTS], f32, tag="iota")
            nc.sync.dma_start(iota_sb[:], iota[:])
            identb_sb = rpool.tile([P, P], bf16, tag="identb")
            nc.sync.dma_start(identb_sb[:], identb[:])
            w1big_sb = rpool.tile([2 * IN_CH + 1, HID], bf16, tag="w1big")
            nc.sync.dma_start(w1big_sb[:], w1big[:])
            w2big_sb = rpool.tile([2 * HID, HID], bf16, tag="w2big")
            nc.sync.dma_start(w2big_sb[:], w2big[:])
            b2rep_sb = rpool.tile([P, HID], f32, tag="b2rep")
            nc.sync.dma_start(b2rep_sb[:], b2rep[:])
            accT_sb = rpool.tile([HID, POOL_SLOTS], f32, tag="accT")
            nc.vector.memset(accT_sb[:], 0.0)
            zrow = rpool.tile([1, CC], bf16, tag="zrow")
            nc.vector.memset(zrow[:], 0.0)

            def gather_group(g, src_full, gpool, spool):
                nb, ps = NB[g], POS[g]
                bb = pos_base[g] // P
                idx_g = spool.tile([P, ps // 16], i16, tag="idxg")
                nc.sync.dma_start(
                    idx_g[:], idx[:, pos_base[g] // 16:pos_base[g + 1] // 16])
                dinv_g = spool.tile([P, 2 * nb], f32, tag="dinvg")
                nc.sync.dma_start(dinv_g[:], dinv[:, 2 * bb:2 * (bb + nb)])
                did_g = dinv_g[:, :nb]
                inv_g = dinv_g[:, nb:]
                xe = gpool.tile([P, nb * XCOL], bf16, tag="xe")
                if no_gather:
                    nc.vector.memset(xe[:, :XCOL], 0.0)
                if not no_gather:
                    for (w, blocks, icol, boff) in segs[g]:
                        L = blocks * P
                        qrr[0] = (qrr[0] + 1) % nq
                        nc.gpsimd.dma_gather(
                            xe[:, boff * XCOL:(boff + blocks) * XCOL].rearrange(
                                "p (b e) -> p b e", e=XCOL),
                            src_full[w * S:(w + 1) * S, :],
                            idx_g[:, icol:icol + L // 16], L, L, XCOL,
                            queue_num=qrr[0])
                return xe, did_g, inv_g

            def seg_sum(nb, xe, did_g, inv_g, F, ps_a, wpool, tag):
                acc_ps = ps_a.tile([F, GT * P], f32, tag="accp" + tag)
                ind0 = None
                if no_ind:
                    ind0 = wpool.tile([P, GT * P], bf16, tag="ind" + tag)
                    nc.vector.memset(ind0[:, :1], 0.0)
                for b in range(nb):
                    if no_ind:
                        ind = ind0
                    else:
                        ind = wpool.tile([P, GT * P], bf16, tag="ind" + tag)
                        nc.vector.tensor_scalar(
                            ind[:], iota_sb[:, :GT * P], did_g[:, b:b + 1],
                            inv_g[:, b:b + 1],
                            op0=mybir.AluOpType.is_equal,
                            op1=mybir.AluOpType.mult)
                    if no_bmm and 0 < b < nb - 1:
                        continue
                    nc.tensor.matmul(
                        acc_ps[:], lhsT=xe[:, b * XCOL:b * XCOL + F],
                        rhs=ind[:], start=(b == 0), stop=(b == nb - 1))
                return acc_ps

            # ---------------- phase 1
            with tc.tile_pool(name="g1", bufs=gbufs) as gpool, \
                 tc.tile_pool(name="s1", bufs=gbufs + 1) as spool, \
                 tc.tile_pool(name="w1", bufs=wbufs) as wpool, \
                 tc.tile_pool(name="pa1", bufs=2, space="PSUM") as ps_a, \
                 tc.tile_pool(name="pz1", bufs=2, space="PSUM") as ps_z, \
                 tc.tile_pool(name="pt1", bufs=2, space="PSUM") as ps_t:
                for g in range(GN):
                    xe, did_g, inv_g = gather_group(g, x_full, gpool, spool)
                    acc_ps = seg_sum(NB[g], xe, did_g, inv_g, IN_CH, ps_a,
                                     wpool, "1")
                    for tau in range(GT):
                        t = g * GT + tau
                        nc.vector.tensor_copy(
                            stacked1[:IN_CH, t * P:(t + 1) * P],
                            acc_ps[:, tau * P:(tau + 1) * P])
                        z_ps = ps_z.tile([P, HID], f32, tag="z")
                        nc.tensor.matmul(
                            z_ps[:], lhsT=stacked1[:, t * P:(t + 1) * P],
                            rhs=w1big_sb[:], start=True, stop=True)
                        h1t = wpool.tile([P, HID], bf16, tag="h1t")
                        nc.scalar.activation(
                            h1t[:], z_ps[:], mybir.ActivationFunctionType.Relu)
                        nc.sync.dma_start(h1_shard[t * P:(t + 1) * P, :],
                                          h1t[:])
                        h1T_ps = ps_t.tile([HID, P], bf16, tag="h1T")
                        nc.tensor.transpose(h1T_ps[:], h1t[:], identb_sb[:])
                        nc.vector.tensor_copy(
                            stacked2[HID:, t * P:(t + 1) * P], h1T_ps[:])

            nc.sync.dma_start(h1_shard[S - 1:S, :], zrow[:])
            if no_cc:
                nc.gpsimd.dma_start(h1_cat[:S, :], h1_shard[:])
            else:
                nc.gpsimd.collective_compute(
                    "AllGather", mybir.AluOpType.bypass, replica_groups=rg,
                    ins=[h1_shard[:].opt()], outs=[h1_cat[:].opt()])
            for w in range(n_cores):
                nc.sync.dma_start(h1_full[w * S:(w + 1) * S, :CC],
                                  h1_cat[w * S:(w + 1) * S, :])

            # ---------------- phase 2 + pooling
            with tc.tile_pool(name="g2", bufs=gbufs) as gpool, \
                 tc.tile_pool(name="s2", bufs=gbufs + 1) as spool, \
                 tc.tile_pool(name="w2", bufs=wbufs) as wpool, \
                 tc.tile_pool(name="pa2", bufs=2, space="PSUM") as ps_a, \
                 tc.tile_pool(name="pz2", bufs=2, space="PSUM") as ps_z, \
                 tc.tile_pool(name="pp2", bufs=2, space="PSUM") as ps_p:
                for g in range(GN):
                    xe, did_g, inv_g = gather_group(g, h1_full, gpool, spool)
                    acc_ps = seg_sum(NB[g], xe, did_g, inv_g, HID, ps_a,
                                     wpool, "2")
                    for tau in range(GT):
                        t = g * GT + tau
                        nc.vector.tensor_copy(
                            stacked2[:HID, t * P:(t + 1) * P],
                            acc_ps[:, tau * P:(tau + 1) * P])
                        z_ps = ps_z.tile([P, HID], f32, tag="z2")
                        nc.tensor.matmul(
                            z_ps[:], lhsT=stacked2[:, t * P:(t + 1) * P],
                            rhs=w2big_sb[:], start=True, stop=True)
                        zb = wpool.tile([P, HID], f32, tag="zb2")
                        nc.vector.tensor_tensor(zb[:], z_ps[:], b2rep_sb[:],
                                                op=mybir.AluOpType.add)
                        h2t = wpool.tile([P, HID], bf16, tag="h2t")
                        nc.scalar.activation(
                            h2t[:], zb[:], mybir.ActivationFunctionType.Relu)
                        if no_pool:
                            continue
                        indp = wpool.tile([P, POOL_SLOTS], bf16, tag="indp")
                        nc.vector.tensor_scalar(
                            indp[:], iota_sb[:], localgf_sb[:, t:t + 1], None,
                            op0=mybir.AluOpType.is_equal)
                        half = POOL_SLOTS // 2
                        for ch in range(2):
                            pp = ps_p.tile([HID, half], f32, tag="pp")
                            nc.tensor.matmul(
                                pp[:], lhsT=h2t[:],
                                rhs=indp[:, ch * half:(ch + 1) * half],
                                start=True, stop=True)
                            a = accT_sb[:, ch * half:(ch + 1) * half]
                            nc.vector.tensor_tensor(a, a, pp[:],
                                                    op=mybir.AluOpType.add)

            nc.sync.dma_start(pool[:], accT_sb[:])

    nc.compile()
    return nc


# ------------------------------------------------------------- kernel builder
def _build_fused(T, K, offs, C, S, n_cores, p1_k1=False, p2_k1=False,
                 no_pool=False, no_cc=False, nq=1):
    AS = n_cores * S
    nc = bacc.Bacc("TRN2", target_bir_lowering=False, debug=False,
                   enable_asserts=False, num_devices=n_cores,
                   num_swdge_queues=nq)
    qi = [0]

    def _q(inst):
        if nq > 1:
            qi[0] = (qi[0] + 1) % nq
            inst.ins.queue = f"qPoolDynamic{qi[0] or ''}"
        return inst
    xs = nc.dram_tensor("xs", [S, IN_CH], bf16, kind="ExternalInput").ap()
    nbr = nc.dram_tensor("nbr", [P, C], i32, kind="ExternalInput").ap()
    selfT1 = nc.dram_tensor("selfT1", [IN_CH, S], bf16,
                            kind="ExternalInput").ap()
    invdeg = nc.dram_tensor("invdeg", [P, T], f32, kind="ExternalInput").ap()
    maskf = nc.dram_tensor("maskf", [P, T], f32, kind="ExternalInput").ap()
    localgf = nc.dram_tensor("localgf", [P, T], f32, kind="ExternalInput").ap()
    iota = nc.dram_tensor("iota", [P, POOL_SLOTS], f32,
                          kind="ExternalInput").ap()
    identf = nc.dram_tensor("identf", [P, P], f32, kind="ExternalInput").ap()
    identb = nc.dram_tensor("identb", [P, P], bf16, kind="ExternalInput").ap()
    w1lT = nc.dram_tensor("w1lT", [IN_CH, HID], f32, kind="ExternalInput").ap()
    w1rT = nc.dram_tensor("w1rT", [IN_CH, HID], bf16,
                          kind="ExternalInput").ap()
    b1rep = nc.dram_tensor("b1rep", [P, HID], f32, kind="ExternalInput").ap()
    w2lT = nc.dram_tensor("w2lT", [HID, HID], f32, kind="ExternalInput").ap()
    w2rT = nc.dram_tensor("w2rT", [HID, HID], bf16, kind="ExternalInput").ap()
    b2rep = nc.dram_tensor("b2rep", [P, HID], f32, kind="ExternalInput").ap()
    pool = nc.dram_tensor("pool", [POOL_SLOTS, HID], f32,
                          kind="ExternalOutput").ap()

    rg = [list(range(n_cores))]
    with tile.TileContext(nc) as tc:
        with tc.tile_pool(name="dramp", bufs=1, space="DRAM") as dpool, \
             tc.tile_pool(name="resident", bufs=1) as rpool:
            x_shard = dpool.tile([S, IN_CH], bf16, tag="x_shard")
            x_full = dpool.tile([AS, IN_CH], bf16, addr_space="Shared",
                                tag="x_full")
            h1_shard = dpool.tile([S, HID], bf16, tag="h1_shard")
            h1_full = dpool.tile([AS, HID], bf16, addr_space="Shared",
                                 tag="h1_full")

            # phase 0: AllGather x shards into the full (permuted) table
            nc.gpsimd.dma_start(x_shard[:], xs[:])
            if no_cc:
                nc.gpsimd.dma_start(x_full[:S, :], x_shard[:])
            else:
                nc.gpsimd.collective_compute(
                    "AllGather", mybir.AluOpType.bypass, replica_groups=rg,
                    ins=[x_shard[:].opt()], outs=[x_full[:].opt()])

            nbr_sb = rpool.tile([P, C], i32, tag="nbr")
            nc.sync.dma_start(nbr_sb[:], nbr[:])
            selfT1_sb = rpool.tile([IN_CH, S], bf16, tag="selfT1")
            nc.sync.dma_start(selfT1_sb[:], selfT1[:])
            invdeg_sb = rpool.tile([P, T], f32, tag="invdeg")
            nc.sync.dma_start(invdeg_sb[:], invdeg[:])
            maskf_sb = rpool.tile([P, T], f32, tag="maskf")
            nc.sync.dma_start(maskf_sb[:], maskf[:])
            localgf_sb = rpool.tile([P, T], f32, tag="localgf")
            nc.sync.dma_start(localgf_sb[:], localgf[:])
            iota_sb = rpool.tile([P, POOL_SLOTS], f32, tag="iota")
            nc.sync.dma_start(iota_sb[:], iota[:])
            identf_sb = rpool.tile([P, P], f32, tag="identf")
            nc.sync.dma_start(identf_sb[:], identf[:])
            identb_sb = rpool.tile([P, P], bf16, tag="identb")
            nc.sync.dma_start(identb_sb[:], identb[:])
            w1lT_sb = rpool.tile([IN_CH, HID], f32, tag="w1lT")
            nc.sync.dma_start(w1lT_sb[:], w1lT[:])
            w1rT_sb = rpool.tile([IN_CH, HID], bf16, tag="w1rT")
            nc.sync.dma_start(w1rT_sb[:], w1rT[:])
            b1rep_sb = rpool.tile([P, HID], f32, tag="b1rep")
            nc.sync.dma_start(b1rep_sb[:], b1rep[:])
            w2lT_sb = rpool.tile([HID, HID], f32, tag="w2lT")
            nc.sync.dma_start(w2lT_sb[:], w2lT[:])
            w2rT_sb = rpool.tile([HID, HID], bf16, tag="w2rT")
            nc.sync.dma_start(w2rT_sb[:], w2rT[:])
            b2rep_sb = rpool.tile([P, HID], f32, tag="b2rep")
            nc.sync.dma_start(b2rep_sb[:], b2rep[:])
            selfT2_sb = rpool.tile([HID, S], bf16, tag="selfT2")
            acc_sb = rpool.tile([P, POOL_CHUNKS * HID], f32, tag="acc")
            nc.vector.memset(acc_sb[:], 0.0)

            # ---------------- phase 1: layer 1 over x_full
            with tc.tile_pool(name="gather1", bufs=3) as gpool, \
                 tc.tile_pool(name="work1", bufs=3) as wpool, \
                 tc.tile_pool(name="ps_t1", bufs=2, space="PSUM") as ps_t, \
                 tc.tile_pool(name="ps_h1", bufs=2, space="PSUM") as ps_h, \
                 tc.tile_pool(name="ps_z1", bufs=2, space="PSUM") as ps_z:
                for t in range(T):
                    kt = 1 if p1_k1 else K[t]
                    g = gpool.tile([P, kt * IN_CH], bf16, tag="g")
                    for k in range(kt):
                        _q(nc.gpsimd.indirect_dma_start(
                            out=g[:, k * IN_CH:(k + 1) * IN_CH],
                            out_offset=None, in_=x_full[:],
                            in_offset=bass.IndirectOffsetOnAxis(
                                ap=nbr_sb[:, offs[t] + k:offs[t] + k + 1],
                                axis=0)))
                    agg = wpool.tile([P, IN_CH], f32, tag="agg")
                    nc.vector.tensor_reduce(
                        agg[:], g[:].rearrange("p (k f) -> p f k", k=kt),
                        axis=mybir.AxisListType.X, op=mybir.AluOpType.add)
                    nc.vector.tensor_scalar_mul(agg[:], agg[:],
                                                invdeg_sb[:, t:t + 1])
                    aggT_ps = ps_t.tile([IN_CH, P], f32, tag="aggT")
                    nc.tensor.transpose(aggT_ps[:], agg[:], identf_sb[:])
                    aggT = wpool.tile([IN_CH, P], f32, tag="aggTs")
                    nc.vector.tensor_copy(aggT[:], aggT_ps[:])
                    z_ps = ps_z.tile([P, HID], f32, tag="z")
                    nc.tensor.matmul(z_ps[:], lhsT=aggT[:], rhs=w1lT_sb[:],
                                     start=True, stop=False)
                    nc.tensor.matmul(z_ps[:],
                                     lhsT=selfT1_sb[:, t * P:(t + 1) * P],
                                     rhs=w1rT_sb[:], start=False, stop=True)
                    zb = wpool.tile([P, HID], f32, tag="zb")
                    nc.vector.tensor_tensor(zb[:], z_ps[:], b1rep_sb[:],
                                            op=mybir.AluOpType.add)
                    nc.vector.tensor_scalar_mul(zb[:], zb[:],
                                                maskf_sb[:, t:t + 1])
                    h1t = wpool.tile([P, HID], bf16, tag="h1t")
                    nc.scalar.activation(h1t[:], zb[:],
                                         mybir.ActivationFunctionType.Relu)
                    nc.sync.dma_start(h1_shard[t * P:(t + 1) * P, :], h1t[:])
                    h1T_ps = ps_h.tile([HID, P], bf16, tag="h1T")
                    nc.tensor.transpose(h1T_ps[:], h1t[:], identb_sb[:])
                    nc.vector.tensor_copy(selfT2_sb[:, t * P:(t + 1) * P],
                                          h1T_ps[:])

            # AllGather h1 across cores
            if no_cc:
                nc.gpsimd.dma_start(h1_full[:S, :], h1_shard[:])
            else:
                nc.gpsimd.collective_compute(
                    "AllGather", mybir.AluOpType.bypass, replica_groups=rg,
                    ins=[h1_shard[:].opt()], outs=[h1_full[:].opt()])

            # ---------------- phase 2: layer 2 over h1_full + pooling
            with tc.tile_pool(name="gather2", bufs=3) as gpool, \
                 tc.tile_pool(name="work2", bufs=3) as wpool, \
                 tc.tile_pool(name="ps_t2", bufs=2, space="PSUM") as ps_t, \
                 tc.tile_pool(name="ps_z2", bufs=2, space="PSUM") as ps_z, \
                 tc.tile_pool(name="ps_p2", bufs=2, space="PSUM") as ps_p:
                for t in range(T):
                    kt = 1 if p2_k1 else K[t]
                    g = gpool.tile([P, kt * HID], bf16, tag="g2")
                    for k in range(kt):
                        _q(nc.gpsimd.indirect_dma_start(
                            out=g[:, k * HID:(k + 1) * HID],
                            out_offset=None, in_=h1_full[:],
                            in_offset=bass.IndirectOffsetOnAxis(
                                ap=nbr_sb[:, offs[t] + k:offs[t] + k + 1],
                                axis=0)))
                    agg = wpool.tile([P, HID], f32, tag="agg2")
                    nc.vector.tensor_reduce(
                        agg[:], g[:].rearrange("p (k f) -> p f k", k=kt),
                        axis=mybir.AxisListType.X, op=mybir.AluOpType.add)
                    nc.vector.tensor_scalar_mul(agg[:], agg[:],
                                                invdeg_sb[:, t:t + 1])
                    aggT_ps = ps_t.tile([HID, P], f32, tag="aggT2")
                    nc.tensor.transpose(aggT_ps[:], agg[:], identf_sb[:])
                    aggT = wpool.tile([HID, P], f32, tag="aggTs2")
                    nc.vector.tensor_copy(aggT[:], aggT_ps[:])
                    z_ps = ps_z.tile([P, HID], f32, tag="z2")
                    nc.tensor.matmul(z_ps[:], lhsT=aggT[:], rhs=w2lT_sb[:],
                                     start=True, stop=False)
                    nc.tensor.matmul(z_ps[:],
                                     lhsT=selfT2_sb[:, t * P:(t + 1) * P],
                                     rhs=w2rT_sb[:], start=False, stop=True)
                    zb = wpool.tile([P, HID], f32, tag="zb2")
                    nc.vector.tensor_tensor(zb[:], z_ps[:], b2rep_sb[:],
                                            op=mybir.AluOpType.add)
                    h2t = wpool.tile([P, HID], bf16, tag="h2t")
                    nc.scalar.activation(h2t[:], zb[:],
                                         mybir.ActivationFunctionType.Relu)
                    if no_pool:
                        continue
                    # one-hot pooling: ind[p, s] = (localg[p] == s)
                    ind = wpool.tile([P, POOL_SLOTS], bf16, tag="ind")
                    nc.vector.tensor_scalar(
                        ind[:], iota_sb[:], localgf_sb[:, t:t + 1], None,
                        op0=mybir.AluOpType.is_equal)
                    for ch in range(POOL_CHUNKS):
                        pp = ps_p.tile([P, HID], f32, tag="pp")
                        nc.tensor.matmul(
                            pp[:], lhsT=ind[:, ch * P:(ch + 1) * P],
                            rhs=h2t[:], start=True, stop=True)
                        a = acc_sb[:, ch * HID:(ch + 1) * HID]
                        nc.vector.tensor_tensor(a, a, pp[:],
                                                op=mybir.AluOpType.add)

            for ch in range(POOL_CHUNKS):
                nc.sync.dma_start(pool[ch * P:(ch + 1) * P, :],
                                  acc_sb[:, ch * HID:(ch + 1) * HID])

    nc.compile()
    return nc


# ---------------------------------------------------------------- jit runner
class _PjrtRunner:
    """Persistent jit(shard_map) executor for a compiled Bass module.

    Same _bass_exec_p machinery run_bass_kernel_spmd uses under axon, but
    the jitted callable and device-resident inputs persist across calls so
    steady-state executions can be timed without re-trace/re-compile or
    host->device traffic.
    """

    def __init__(self, nc, n_cores):
        import jax
        from jax.experimental.shard_map import shard_map
        from jax.sharding import Mesh, NamedSharding, PartitionSpec
        from concourse import bass2jax

        bass2jax.install_neuronx_cc_hook()
        self.jax = jax
        self.nc = nc
        self.n_cores = n_cores
        partition_name = (nc.partition_id_tensor.name
                          if nc.partition_id_tensor else None)
        in_names, out_names, out_avals, zero_shapes = [], [], [], []
        for alloc in nc.m.functions[0].allocations:
            if not isinstance(alloc, mybir.MemoryLocationSet):
                continue
            name = alloc.memorylocations[0].name
            if alloc.kind == "ExternalInput":
                if name != partition_name:
                    in_names.append(name)
            elif alloc.kind == "ExternalOutput":
                shape = tuple(alloc.tensor_shape)
                dtype = mybir.dt.np(alloc.dtype)
                out_names.append(name)
                out_avals.append(jax.core.ShapedArray(shape, dtype))
                zero_shapes.append((shape, dtype))
        self.in_names = list(in_names)
        self.out_names = out_names
        self.zero_shapes = zero_shapes
        n_params = len(in_names)
        n_outs = len(out_names)
        all_names = in_names + out_names
        if partition_name is not None:
            all_names.append(partition_name)
        donate = tuple(range(n_params, n_params + n_outs))

        def _body(*args):
            operands = list(args)
            if partition_name is not None:
                operands.append(bass2jax.partition_id_tensor())
            outs = bass2jax._bass_exec_p.bind(
                *operands,
                out_avals=tuple(out_avals),
                in_names=tuple(all_names),
                out_names=tuple(out_names),
                lowering_input_output_aliases=(),
                sim_require_finite=True,
                sim_require_nnan=True,
                nc=nc,
            )
            return tuple(outs)

        devices = jax.devices()[:n_cores]
        assert len(devices) == n_cores
        self.mesh = Mesh(np.asarray(devices), ("core",))
        self.sharding = NamedSharding(self.mesh, PartitionSpec("core"))
        in_specs = (PartitionSpec("core"),) * (n_params + n_outs)
        out_specs = (PartitionSpec("core"),) * n_outs
        self._fn = jax.jit(
            shard_map(_body, mesh=self.mesh, in_specs=in_specs,
                      out_specs=out_specs, check_rep=False),
            donate_argnums=donate, keep_unused=True)
        self._dev_inputs = None

    def put_inputs(self, in_maps):
        """Concatenate per-core inputs on axis 0 and place on device."""
        jax = self.jax
        self._dev_inputs = []
        for name in self.in_names:
            arr = np.concatenate([np.asarray(m[name]) for m in in_maps],
                                 axis=0)
            self._dev_inputs.append(
                jax.device_put(arr, self.sharding))
        jax.block_until_ready(self._dev_inputs)

    def _zeros(self):
        jax = self.jax
        zs = [jax.device_put(
                  np.zeros((self.n_cores * s[0], *s[1:]), d), self.sharding)
              for s, d in self.zero_shapes]
        jax.block_until_ready(zs)
        return zs

    def run(self, zeros=None):
        jax = self.jax
        if zeros is None:
            zeros = self._zeros()
        outs = self._fn(*self._dev_inputs, *zeros)
        jax.block_until_ready(outs)
        return outs

    def results(self, outs):
        per_core = []
        for c in range(self.n_cores):
            m = {}
            for i, name in enumerate(self.out_names):
                shape, _ = self.zero_shapes[i]
                m[name] = np.asarray(outs[i]).reshape(
                    self.n_cores, *shape)[c]
            per_core.append(m)
        return per_core

    def time_runs(self, reps=3):
        zero_sets = [self._zeros() for _ in range(reps)]
        times = []
        for zs in zero_sets:
            t0 = time.perf_counter()
            outs = self._fn(*self._dev_inputs, *zs)
            self.jax.block_until_ready(outs)
            times.append(time.perf_counter() - t0)
        return times

    def time_pipelined(self, reps=10):
        """Issue `reps` executions back-to-back (async dispatch), block once.

        Per-run time = total / reps: dispatch latency overlaps execution, so
        this converges to the device execution+collective time per run.
        """
        zero_sets = [self._zeros() for _ in range(reps)]
        all_outs = []
        t0 = time.perf_counter()
        for zs in zero_sets:
            all_outs.append(self._fn(*self._dev_inputs, *zs))
        self.jax.block_until_ready(all_outs)
        return (time.perf_counter() - t0) / reps


# -------------------------------------------------------------------- kernel
def _kernel_impl(x, edge_index, batch, W1_l, b1, W1_r, W2_l, b2, W2_r,
                 W_lin, b_lin, n_nodes, n_graphs, n_cores, _timing=None,
                 mode="eg"):
    x = np.asarray(x, dtype=np.float32)
    W1_l = np.asarray(W1_l, np.float32)
    W1_r = np.asarray(W1_r, np.float32)
    W2_l = np.asarray(W2_l, np.float32)
    W2_r = np.asarray(W2_r, np.float32)
    b1 = np.asarray(b1, np.float32)
    b2 = np.asarray(b2, np.float32)
    W_lin = np.asarray(W_lin, np.float32)
    b_lin = np.asarray(b_lin, np.float32)

    iota = np.broadcast_to(
        np.arange(POOL_SLOTS, dtype=np.float32), (P, POOL_SLOTS)).copy()
    identb = np.eye(P).astype(bfl)
    b2rep = np.broadcast_to(b2, (P, HID)).copy()

    t0 = time.time()
    if mode == "eg":
        pp = _prep_eg(edge_index, batch, n_nodes, n_graphs, n_cores)
        T, S = pp["T"], pp["S"]
        t_prep = time.time() - t0

        t0 = time.time()
        nc = _build_eg(T, S, pp["GN"], pp["NB"], pp["POS"], pp["pos_base"],
                       pp["segs"], n_cores)
        t_build = time.time() - t0

        w1big = np.zeros((2 * IN_CH + 1, HID), np.float32)
        w1big[:IN_CH] = W1_l.T
        w1big[IN_CH:2 * IN_CH] = W1_r.T
        w1big[2 * IN_CH] = b1
        w2big = np.concatenate([W2_l.T, W2_r.T], axis=0)
        in_maps = []
        for ci in pp["cores"]:
            xs = np.zeros((S, 64), bfl)
            xs[:ci["n"], :IN_CH] = x[ci["ids"]]
            in_maps.append(dict(
                xs=xs,
                selfT1=np.ascontiguousarray(xs[:, :IN_CH].T),
                idx=ci["idx"], dinv=ci["dinv"],
                localgf=ci["localgf"], iota=iota, identb=identb,
                w1big=w1big.astype(bfl), w2big=w2big.astype(bfl),
                b2rep=b2rep))
    else:
        pp = _prep(edge_index, batch, n_nodes, n_graphs, n_cores)
        T, S, K, offs, C = pp["T"], pp["S"], pp["K"], pp["offs"], pp["C"]
        t_prep = time.time() - t0

        t0 = time.time()
        nc = _build_fused(T, K, offs, C, S, n_cores)
        t_build = time.time() - t0

        identf = np.eye(P, dtype=np.float32)
        b1rep = np.broadcast_to(b1, (P, HID)).copy()
        in_maps = []
        for ci in pp["cores"]:
            xs = np.zeros((S, IN_CH), bfl)
            xs[:ci["n"]] = x[ci["ids"]]
            in_maps.append(dict(
                xs=xs, nbr=ci["nbr"],
                selfT1=np.ascontiguousarray(xs.T),
                invdeg=ci["invdeg"], maskf=ci["maskf"], localgf=ci["localgf"],
                iota=iota, identf=identf, identb=identb,
                w1lT=np.ascontiguousarray(W1_l.T),
                w1rT=np.ascontiguousarray(W1_r.T).astype(bfl),
                b1rep=b1rep,
                w2lT=np.ascontiguousarray(W2_l.T),
                w2rT=np.ascontiguousarray(W2_r.T).astype(bfl),
                b2rep=b2rep))

    t0 = time.time()
    runner = _PjrtRunner(nc, n_cores)
    runner.put_inputs(in_maps)
    t_put = time.time() - t0

    t0 = time.time()
    outs = runner.run()  # first call compiles (NEFF via neuronx_cc hook)
    t_first = time.time() - t0
    res = runner.results(outs)

    # host: divide pooled sums by graph node counts; final linear
    pool = np.zeros((n_graphs, HID), np.float32)
    for c, (ci, r) in enumerate(zip(pp["cores"], res)):
        ng = ci["ghi"] - ci["glo"]
        rp = r["pool"]
        if rp.shape[0] == HID:  # eg kernel outputs [HID, POOL_SLOTS]
            rp = np.ascontiguousarray(rp.T)
        pool[ci["glo"]:ci["ghi"]] = rp[:ng]
    gcnt = np.maximum(pp["gcnt"], 1).astype(np.float32)
    pooled = pool / gcnt[:, None]
    out = (pooled @ W_lin.T + b_lin).astype(np.float32)

    if _timing is not None:
        times = runner.time_runs(reps=2)
        tot20 = runner.time_pipelined(reps=20) * 20
        tot40 = runner.time_pipelined(reps=40) * 40
        _timing.update(prep=t_prep, build=t_build, put=t_put,
                       first=t_first, reps=times,
                       piped40=tot40 / 40, marginal=(tot40 - tot20) / 20,
                       exec_ns=tot40 / 40 * 1e9)
    return out


def kernel(x, edge_index, batch, W1_l, b1, W1_r, W2_l, b2, W2_r, W_lin,
           b_lin, _timing=None):
    return _kernel_impl(x, edge_index, batch, W1_l, b1, W1_r, W2_l, b2,
                        W2_r, W_lin, b_lin, N_NODES, N_GRAPHS, N_CORES,
                        _timing=_timing)


# revision 32
# speedup vs baseline: 1.2134x; 1.1691x over previous
"""Trainium2 fused Bass kernel for JetGNN (2-layer SAGEConv + global mean pool).

Single fused NEFF, SPMD x8, graph-aligned node sharding (batch is sorted, so
whole jets stay on one core and pooling never crosses cores). Message passing
is edge-major around the SWDGE dma_gather primitive:

  - Host: per-core nodes are degree-sorted; edges are bucketed per
    (4-tile dst group, source-core window), sorted, padded to a core-uniform
    layout; window-local int16 gather indices + per-edge (dst-slot, 1/deg)
    tables; pad edges point at a guaranteed-zero row.
  - Device:  AllGather x shards (bf16, compact [S,64]) -> spread to 256B-pitch
    rows (dma_gather granularity).  Per group: dma_gather the edges' source
    rows (4 SWDGE queues round-robin), build one-hot indicators
    (is_equal vs iota, scaled by 1/deg via the fused second scalar op) and
    segment-sum via PSUM-accumulated matmuls -> mean^T directly (no PE
    transpose).  The layer matmul stacks [mean; self; ones] so W_l, W_r and
    the bias fold into one PE op.  h1 -> DRAM, AllGather, spread, repeat for
    layer 2; global pooling via two transposed one-hot matmuls per tile into
    a resident SBUF accumulator.
  - Host: divide pooled sums by graph node counts, final 64->2 linear.

Timing: a persistent jit(shard_map) runner (the same _bass_exec_p machinery
run_bass_kernel_spmd uses under axon) compiles once, keeps inputs on device,
and reports steady-state per-run time from 40 pipelined executions
(block_until_ready once at the end; the axon tunnel adds ~70ms dispatch
latency per blocking call, which pipelining amortizes away).
"""

import math
import time

import numpy as np
import ml_dtypes

import concourse.bass as bass
import concourse.tile as tile
import concourse.mybir as mybir
from concourse import bacc

N_NODES = 200000
N_GRAPHS = 4000
N_CORES = 8
IN_CH = 32
HID = 64
P = 128
POOL_SLOTS = 640  # per-core graph slots (~500 graphs/core); last slot = dump
DUMP_SLOT = POOL_SLOTS - 1
POOL_CHUNKS = POOL_SLOTS // P

f32 = mybir.dt.float32
bf16 = mybir.dt.bfloat16
i32 = mybir.dt.int32
bfl = ml_dtypes.bfloat16


# ----------------------------------------------------------------- host prep
def _prep(edge_index, batch, n_nodes, n_graphs, n_cores):
    """Vectorized host prep. Returns layout + per-core tables."""
    src = np.asarray(edge_index[0]).astype(np.int64)
    dst = np.asarray(edge_index[1]).astype(np.int64)
    batch = np.asarray(batch).astype(np.int64)
    deg = np.bincount(dst, minlength=n_nodes).astype(np.int64)

    # CSR by dst
    order = np.argsort(dst, kind="stable")
    src_sorted = src[order]
    rowptr = np.zeros(n_nodes + 1, dtype=np.int64)
    np.cumsum(deg, out=rowptr[1:])

    # graph-aligned core boundaries (batch is sorted by graph id)
    gcnt = np.bincount(batch, minlength=n_graphs)
    gends = np.cumsum(gcnt)
    targets = (np.arange(1, n_cores) * n_nodes) // n_cores
    gb = np.searchsorted(gends, targets)
    graph_bounds = np.concatenate([[0], gb + 1, [n_graphs]])
    node_bounds = np.concatenate([[0], gends[graph_bounds[1:-1] - 1], [n_nodes]])

    cores = []
    for c in range(n_cores):
        lo, hi = int(node_bounds[c]), int(node_bounds[c + 1])
        perm = np.argsort(-deg[lo:hi], kind="stable")
        ids = np.arange(lo, hi)[perm]
        glo, ghi = int(graph_bounds[c]), int(graph_bounds[c + 1])
        assert ghi - glo <= DUMP_SLOT
        cores.append(dict(ids=ids, glo=glo, ghi=ghi, n=hi - lo))

    T = max(1, math.ceil(max(ci["n"] for ci in cores) / P))
    S = T * P
    # per-tile K: max over cores of max degree within the tile
    K = np.ones(T, dtype=np.int64)
    for ci in cores:
        d = np.pad(deg[ci["ids"]], (0, S - ci["n"]))
        K = np.maximum(K, d.reshape(T, P).max(axis=1))
    offs = np.zeros(T + 1, dtype=np.int64)
    np.cumsum(K, out=offs[1:])
    C = int(offs[-1])

    # global position of each node in the AllGathered [n_cores*S] layout
    global_pos = np.empty(n_nodes, dtype=np.int64)
    for c, ci in enumerate(cores):
        global_pos[ci["ids"]] = c * S + np.arange(ci["n"])
    pad_core = next(c for c, ci in enumerate(cores) if ci["n"] < S)
    pad_slot = pad_core * S + cores[pad_core]["n"]  # guaranteed all-zero row

    for c, ci in enumerate(cores):
        ids, n = ci["ids"], ci["n"]
        deg_c = deg[ids]
        tot = int(deg_c.sum())
        # edge-wise coordinates: node at position pos -> tile t, row p
        cum0 = np.zeros(n, dtype=np.int64)
        np.cumsum(deg_c[:-1], out=cum0[1:])
        e_node = np.repeat(np.arange(n), deg_c)  # position of dst node
        j = np.arange(tot) - np.repeat(cum0, deg_c)  # slot within nbr list
        e_idx = np.repeat(rowptr[ids], deg_c) + j  # CSR edge index
        t_e = e_node // P
        p_e = e_node % P
        col = offs[t_e] + j
        nbr = np.full(P * C, pad_slot, dtype=np.int32)
        nbr[p_e * C + col] = global_pos[src_sorted[e_idx]].astype(np.int32)

        def _tileize(vals, pad_val, dtype):
            a = np.full(S, pad_val, dtype=dtype)
            a[:n] = vals
            return np.ascontiguousarray(a.reshape(T, P).T)

        invdeg = _tileize(1.0 / np.maximum(deg_c, 1), 0.0, np.float32)
        maskf = _tileize(np.ones(n), 0.0, np.float32)
        localgf = _tileize(batch[ids] - ci["glo"], DUMP_SLOT, np.float32)
        ci.update(nbr=nbr.reshape(P, C), invdeg=invdeg, maskf=maskf,
                  localgf=localgf)

    return dict(cores=cores, T=T, S=S, K=K.tolist(), offs=offs.tolist(), C=C,
                node_bounds=node_bounds, graph_bounds=graph_bounds,
                gcnt=gcnt, batch=batch)


# ----------------------------------------------------- edge-major host prep
GT = 4  # dst tiles per group (512 slots -> one 2KB PSUM bank at f32)
GATHER_CHUNK = 512  # idxs per dma_gather (best 4-queue balance measured)
SENT = 4096.0  # dstid sentinel for pad edges (matches no iota column)


def _prep_eg(edge_index, batch, n_nodes, n_graphs, n_cores):
    """Edge-major prep for the dma_gather kernel.

    Edges are bucketed per (dst-tile-group, source-core window), sorted, and
    padded to a single structure shared by all cores (max over cores), so the
    SPMD program is uniform. Indices are window-local (int16) positions into
    the AllGathered [n_cores*S] node table.
    """
    src = np.asarray(edge_index[0]).astype(np.int64)
    dst = np.asarray(edge_index[1]).astype(np.int64)
    batch = np.asarray(batch).astype(np.int64)
    deg = np.bincount(dst, minlength=n_nodes).astype(np.int64)

    gcnt = np.bincount(batch, minlength=n_graphs)
    gends = np.cumsum(gcnt)
    targets = (np.arange(1, n_cores) * n_nodes) // n_cores
    gb = np.searchsorted(gends, targets)
    graph_bounds = np.concatenate([[0], gb + 1, [n_graphs]])
    node_bounds = np.concatenate([[0], gends[graph_bounds[1:-1] - 1],
                                  [n_nodes]])

    cores = []
    for c in range(n_cores):
        lo, hi = int(node_bounds[c]), int(node_bounds[c + 1])
        perm = np.argsort(-deg[lo:hi], kind="stable")
        ids = np.arange(lo, hi)[perm]
        glo, ghi = int(graph_bounds[c]), int(graph_bounds[c + 1])
        assert ghi - glo <= DUMP_SLOT
        slot = np.empty(hi - lo, dtype=np.int64)
        slot[perm] = np.arange(hi - lo)
        cores.append(dict(ids=ids, glo=glo, ghi=ghi, n=hi - lo, lo=lo, hi=hi,
                          slot=slot))

    max_n = max(ci["n"] for ci in cores)
    T = math.ceil((max_n + 1) / P)  # +1: every core keeps a zero pad row
    T = ((T + GT - 1) // GT) * GT
    S = T * P
    GN = T // GT
    GS = GT * P  # slots per group

    global_pos = np.empty(n_nodes, dtype=np.int64)
    for c, ci in enumerate(cores):
        global_pos[ci["ids"]] = c * S + np.arange(ci["n"])

    # per-core edge buckets -> uniform (max over cores) segment sizes
    cnts = np.zeros((n_cores, GN, n_cores), dtype=np.int64)
    percore = []
    for c, ci in enumerate(cores):
        emask = (dst >= ci["lo"]) & (dst < ci["hi"])
        ed, es = dst[emask], src[emask]
        dslot = ci["slot"][ed - ci["lo"]]
        sg = global_pos[es]
        g_e = dslot // GS
        w_e = sg // S
        order = np.lexsort((dslot, w_e, g_e))
        percore.append(dict(
            g=g_e[order], w=w_e[order],
            loc=(sg[order] % S).astype(np.int16),
            did=(dslot[order] - g_e[order] * GS).astype(np.float32),
            inv=(1.0 / deg[ed[order]]).astype(np.float32)))
        np.add.at(cnts[c], (g_e, w_e), 1)

    Lgw = ((cnts.max(axis=0) + P - 1) // P) * P  # [GN, n_cores]
    # guarantee >= 1 block per group (isolated/pad-only groups)
    empty_g = Lgw.sum(axis=1) == 0
    Lgw[empty_g, 0] = P
    NB = (Lgw.sum(axis=1) // P).astype(np.int64)  # blocks per group
    POS = NB * P
    seg_base = np.zeros((GN, n_cores), dtype=np.int64)
    pos_base = np.zeros(GN + 1, dtype=np.int64)
    for g in range(GN):
        pos_base[g + 1] = pos_base[g] + POS[g]
        seg_base[g] = pos_base[g] + np.concatenate(
            [[0], np.cumsum(Lgw[g][:-1])])
    POSTOT = int(pos_base[-1])
    NBTOT = POSTOT // P
    # static segments for codegen: per group, (window, n_blocks, idx col/16,
    # out block offset) — chunked to <= 2048 idxs
    segs = []
    for g in range(GN):
        sg_list = []
        for w in range(n_cores):
            L = int(Lgw[g][w])
            off = int(seg_base[g][w] - pos_base[g])
            while L > 0:
                chunk = min(L, GATHER_CHUNK)
                sg_list.append((w, chunk // P, off // 16, off // P))
                off += chunk
                L -= chunk
        segs.append(sg_list)

    for c, ci in enumerate(cores):
        pc = percore[c]
        cell = pc["g"] * n_cores + pc["w"]
        cell_counts = np.bincount(cell, minlength=GN * n_cores)
        cell_start = np.concatenate([[0], np.cumsum(cell_counts)[:-1]])
        rank = np.arange(len(cell)) - cell_start[cell]
        tgt = seg_base.reshape(-1)[cell] + rank
        idxf = np.full(POSTOT, S - 1, dtype=np.int16)
        didf = np.full(POSTOT, SENT, dtype=np.float32)
        invf = np.zeros(POSTOT, dtype=np.float32)
        idxf[tgt] = pc["loc"]
        didf[tgt] = pc["did"]
        invf[tgt] = pc["inv"]
        ci["idx"] = np.tile(np.ascontiguousarray(
            idxf.reshape(-1, 16).T), (n_cores, 1))
        ci["dstid"] = np.ascontiguousarray(didf.reshape(-1, P).T)
        ci["invde"] = np.ascontiguousarray(invf.reshape(-1, P).T)
        nbtot = POSTOT // P
        dinv = np.empty((P, 2 * nbtot), np.float32)
        for g in range(GN):
            bb = pos_base[g] // P
            nb = int(NB[g])
            dinv[:, 2 * bb:2 * bb + nb] = ci["dstid"][:, bb:bb + nb]
            dinv[:, 2 * bb + nb:2 * (bb + nb)] = ci["invde"][:, bb:bb + nb]
        ci["dinv"] = dinv
        lg = np.full(S, DUMP_SLOT, dtype=np.float32)
        lg[:ci["n"]] = batch[ci["ids"]] - ci["glo"]
        ci["localgf"] = np.ascontiguousarray(lg.reshape(T, P).T)

    # in-group slice offsets for per-group table streams
    return dict(cores=cores, T=T, S=S, GN=GN, NB=NB.tolist(),
                POS=POS.tolist(), pos_base=pos_base.tolist(), segs=segs,
                gcnt=gcnt, batch=batch)


# ------------------------------------------------- edge-major kernel builder
def _build_eg(T, S, GN, NB, POS, pos_base, segs, n_cores, no_gather=False,
              with_lib=True, no_cc=False, no_pool=False, no_ind=False,
              no_bmm=False, scratch=None, nq=4, gbufs=2, wbufs=3):
    from concourse.library_config import mlp as mlp_lib

    AS = n_cores * S
    XCOL = 128  # gathered row width (256B at bf16, dma_gather granularity)
    CC = 64  # compact row width for DRAM tables / collectives
    NBTOT = pos_base[-1] // P
    kw = dict(dynamic_dma_scratch_size=scratch) if scratch else {}
    nc = bacc.Bacc("TRN2", target_bir_lowering=False, debug=False,
                   enable_asserts=False, num_devices=n_cores,
                   num_swdge_queues=nq, **kw)
    i16 = mybir.dt.int16
    qrr = [0]
    xs = nc.dram_tensor("xs", [S, CC], bf16, kind="ExternalInput").ap()
    selfT1 = nc.dram_tensor("selfT1", [IN_CH, S], bf16,
                            kind="ExternalInput").ap()
    idx = nc.dram_tensor("idx", [P, pos_base[-1] // 16], i16,
                         kind="ExternalInput").ap()
    dinv = nc.dram_tensor("dinv", [P, 2 * NBTOT], f32,
                          kind="ExternalInput").ap()
    localgf = nc.dram_tensor("localgf", [P, T], f32, kind="ExternalInput").ap()
    iota = nc.dram_tensor("iota", [P, POOL_SLOTS], f32,
                          kind="ExternalInput").ap()
    identb = nc.dram_tensor("identb", [P, P], bf16, kind="ExternalInput").ap()
    w1big = nc.dram_tensor("w1big", [2 * IN_CH + 1, HID], bf16,
                           kind="ExternalInput").ap()
    w2big = nc.dram_tensor("w2big", [2 * HID, HID], bf16,
                           kind="ExternalInput").ap()
    b2rep = nc.dram_tensor("b2rep", [P, HID], f32, kind="ExternalInput").ap()
    pool = nc.dram_tensor("pool", [HID, POOL_SLOTS], f32,
                          kind="ExternalOutput").ap()

    rg = [list(range(n_cores))]
    with tile.TileContext(nc) as tc:
        if with_lib:
            nc.gpsimd.load_library(mlp_lib)
        with tc.tile_pool(name="dramp", bufs=1, space="DRAM") as dpool, \
             tc.tile_pool(name="resident", bufs=1) as rpool:
            x_shard = dpool.tile([S, CC], bf16, tag="x_shard")
            x_cat = dpool.tile([AS, CC], bf16, addr_space="Shared",
                               tag="x_cat")
            x_full = dpool.tile([AS, XCOL], bf16, tag="x_full")
            h1_shard = dpool.tile([S, CC], bf16, tag="h1_shard")
            h1_cat = dpool.tile([AS, CC], bf16, addr_space="Shared",
                                tag="h1_cat")
            h1_full = dpool.tile([AS, XCOL], bf16, tag="h1_full")

            nc.gpsimd.dma_start(x_shard[:], xs[:])
            if no_cc:
                nc.gpsimd.dma_start(x_cat[:S, :], x_shard[:])
            else:
                nc.gpsimd.collective_compute(
                    "AllGather", mybir.AluOpType.bypass, replica_groups=rg,
                    ins=[x_shard[:].opt()], outs=[x_cat[:].opt()])
            # spread compact rows to 256B pitch for dma_gather
            # (split: DMA AP dim counts are 16-bit)
            for w in range(n_cores):
                nc.sync.dma_start(x_full[w * S:(w + 1) * S, :CC],
                                  x_cat[w * S:(w + 1) * S, :])

            stacked1 = rpool.tile([2 * IN_CH + 1, T * P], bf16,
                                  tag="stacked1")
            nc.sync.dma_start(stacked1[IN_CH:2 * IN_CH, :], selfT1[:])
            nc.vector.memset(stacked1[2 * IN_CH:2 * IN_CH + 1, :], 1.0)
            stacked2 = rpool.tile([2 * HID, T * P], bf16, tag="stacked2")
            localgf_sb = rpool.tile([P, T], f32, tag="localgf")
            nc.sync.dma_start(localgf_sb[:], localgf[:])
            iota_sb = rpool.tile([P, POOL_SLOTS], f32, tag="iota")
            nc.sync.dma_start(iota_sb[:], iota[:])
            identb_sb = rpool.tile([P, P], bf16, tag="identb")
            nc.sync.dma_start(identb_sb[:], identb[:])
            w1big_sb = rpool.tile([2 * IN_CH + 1, HID], bf16, tag="w1big")
            nc.sync.dma_start(w1big_sb[:], w1big[:])
            w2big_sb = rpool.tile([2 * HID, HID], bf16, tag="w2big")
            nc.sync.dma_start(w2big_sb[:], w2big[:])
            b2rep_sb = rpool.tile([P, HID], f32, tag="b2rep")
            nc.sync.dma_start(b2rep_sb[:], b2rep[:])
            accT_sb = rpool.tile([HID, POOL_SLOTS], f32, tag="accT")
            nc.vector.memset(accT_sb[:], 0.0)
            zrow = rpool.tile([1, CC], bf16, tag="zrow")
            nc.vector.memset(zrow[:], 0.0)

            def gather_group(g, src_full, gpool, spool):
                nb, ps = NB[g], POS[g]
                bb = pos_base[g] // P
                idx_g = spool.tile([P, ps // 16], i16, tag="idxg")
                nc.sync.dma_start(
                    idx_g[:], idx[:, pos_base[g] // 16:pos_base[g + 1] // 16])
                dinv_g = spool.tile([P, 2 * nb], f32, tag="dinvg")
                nc.sync.dma_start(dinv_g[:], dinv[:, 2 * bb:2 * (bb + nb)])
                did_g = dinv_g[:, :nb]
                inv_g = dinv_g[:, nb:]
                xe = gpool.tile([P, nb * XCOL], bf16, tag="xe")
                if no_gather:
                    nc.vector.memset(xe[:, :XCOL], 0.0)
                if not no_gather:
                    for (w, blocks, icol, boff) in segs[g]:
                        L = blocks * P
                        qrr[0] = (qrr[0] + 1) % nq
                        nc.gpsimd.dma_gather(
                            xe[:, boff * XCOL:(boff + blocks) * XCOL].rearrange(
                                "p (b e) -> p b e", e=XCOL),
                            src_full[w * S:(w + 1) * S, :],
                            idx_g[:, icol:icol + L // 16], L, L, XCOL,
                            queue_num=qrr[0])
                return xe, did_g, inv_g

            def seg_sum(nb, xe, did_g, inv_g, F, ps_a, wpool, tag):
                acc_ps = ps_a.tile([F, GT * P], f32, tag="accp" + tag)
                ind0 = None
                if no_ind:
                    ind0 = wpool.tile([P, GT * P], bf16, tag="ind" + tag)
                    nc.vector.memset(ind0[:, :1], 0.0)
                for b in range(nb):
                    if no_ind:
                        ind = ind0
                    else:
                        ind = wpool.tile([P, GT * P], bf16, tag="ind" + tag)
                        nc.vector.tensor_scalar(
                            ind[:], iota_sb[:, :GT * P], did_g[:, b:b + 1],
                            inv_g[:, b:b + 1],
                            op0=mybir.AluOpType.is_equal,
                            op1=mybir.AluOpType.mult)
                    if no_bmm and 0 < b < nb - 1:
                        continue
                    nc.tensor.matmul(
                        acc_ps[:], lhsT=xe[:, b * XCOL:b * XCOL + F],
                        rhs=ind[:], start=(b == 0), stop=(b == nb - 1))
                return acc_ps

            # ---------------- phase 1
            with tc.tile_pool(name="g1", bufs=gbufs) as gpool, \
                 tc.tile_pool(name="s1", bufs=gbufs + 1) as spool, \
                 tc.tile_pool(name="w1", bufs=wbufs) as wpool, \
                 tc.tile_pool(name="pa1", bufs=2, space="PSUM") as ps_a, \
                 tc.tile_pool(name="pz1", bufs=2, space="PSUM") as ps_z, \
                 tc.tile_pool(name="pt1", bufs=2, space="PSUM") as ps_t:
                for g in range(GN):
                    xe, did_g, inv_g = gather_group(g, x_full, gpool, spool)
                    acc_ps = seg_sum(NB[g], xe, did_g, inv_g, IN_CH, ps_a,
                                     wpool, "1")
                    for tau in range(GT):
                        t = g * GT + tau
                        nc.vector.tensor_copy(
                            stacked1[:IN_CH, t * P:(t + 1) * P],
                            acc_ps[:, tau * P:(tau + 1) * P])
                        z_ps = ps_z.tile([P, HID], f32, tag="z")
                        nc.tensor.matmul(
                            z_ps[:], lhsT=stacked1[:, t * P:(t + 1) * P],
                            rhs=w1big_sb[:], start=True, stop=True)
                        h1t = wpool.tile([P, HID], bf16, tag="h1t")
                        nc.scalar.activation(
                            h1t[:], z_ps[:], mybir.ActivationFunctionType.Relu)
                        nc.sync.dma_start(h1_shard[t * P:(t + 1) * P, :],
                                          h1t[:])
                        h1T_ps = ps_t.tile([HID, P], bf16, tag="h1T")
                        nc.tensor.transpose(h1T_ps[:], h1t[:], identb_sb[:])
                        nc.vector.tensor_copy(
                            stacked2[HID:, t * P:(t + 1) * P], h1T_ps[:])

            nc.sync.dma_start(h1_shard[S - 1:S, :], zrow[:])
            if no_cc:
                nc.gpsimd.dma_start(h1_cat[:S, :], h1_shard[:])
            else:
                nc.gpsimd.collective_compute(
                    "AllGather", mybir.AluOpType.bypass, replica_groups=rg,
                    ins=[h1_shard[:].opt()], outs=[h1_cat[:].opt()])
            for w in range(n_cores):
                nc.sync.dma_start(h1_full[w * S:(w + 1) * S, :CC],
                                  h1_cat[w * S:(w + 1) * S, :])

            # ---------------- phase 2 + pooling
            with tc.tile_pool(name="g2", bufs=gbufs) as gpool, \
                 tc.tile_pool(name="s2", bufs=gbufs + 1) as spool, \
                 tc.tile_pool(name="w2", bufs=wbufs) as wpool, \
                 tc.tile_pool(name="pa2", bufs=2, space="PSUM") as ps_a, \
                 tc.tile_pool(name="pz2", bufs=2, space="PSUM") as ps_z, \
                 tc.tile_pool(name="pp2", bufs=2, space="PSUM") as ps_p:
                for g in range(GN):
                    xe, did_g, inv_g = gather_group(g, h1_full, gpool, spool)
                    acc_ps = seg_sum(NB[g], xe, did_g, inv_g, HID, ps_a,
                                     wpool, "2")
                    for tau in range(GT):
                        t = g * GT + tau
                        nc.vector.tensor_copy(
                            stacked2[:HID, t * P:(t + 1) * P],
                            acc_ps[:, tau * P:(tau + 1) * P])
                        z_ps = ps_z.tile([P, HID], f32, tag="z2")
                        nc.tensor.matmul(
                            z_ps[:], lhsT=stacked2[:, t * P:(t + 1) * P],
                            rhs=w2big_sb[:], start=True, stop=True)
                        zb = wpool.tile([P, HID], f32, tag="zb2")
                        nc.vector.tensor_tensor(zb[:], z_ps[:], b2rep_sb[:],
                                                op=mybir.AluOpType.add)
                        h2t = wpool.tile([P, HID], bf16, tag="h2t")
                        nc.scalar.activation(
                            h2t[:], zb[:], mybir.ActivationFunctionType.Relu)
                        if no_pool:
                            continue
                        indp = wpool.tile([P, POOL_SLOTS], bf16, tag="indp")
                        nc.vector.tensor_scalar(
                            indp[:], iota_sb[:], localgf_sb[:, t:t + 1], None,
                            op0=mybir.AluOpType.is_equal)
                        half = POOL_SLOTS // 2
                        for ch in range(2):
                            pp = ps_p.tile([HID, half], f32, tag="pp")
                            nc.tensor.matmul(
                                pp[:], lhsT=h2t[:],
                                rhs=indp[:, ch * half:(ch + 1) * half],
                                start=True, stop=True)
                            a = accT_sb[:, ch * half:(ch + 1) * half]
                            nc.vector.tensor_tensor(a, a, pp[:],
                                                    op=mybir.AluOpType.add)

            nc.sync.dma_start(pool[:], accT_sb[:])

    nc.compile()
    return nc


# ------------------------------------------------------------- kernel builder
def _build_fused(T, K, offs, C, S, n_cores, p1_k1=False, p2_k1=False,
                 no_pool=False, no_cc=False, nq=1):
    AS = n_cores * S
    nc = bacc.Bacc("TRN2", target_bir_lowering=False, debug=False,
                   enable_asserts=False, num_devices=n_cores,
                   num_swdge_queues=nq)
    qi = [0]

    def _q(inst):
        if nq > 1:
            qi[0] = (qi[0] + 1) % nq
            inst.ins.queue = f"qPoolDynamic{qi[0] or ''}"
        return inst
    xs = nc.dram_tensor("xs", [S, IN_CH], bf16, kind="ExternalInput").ap()
    nbr = nc.dram_tensor("nbr", [P, C], i32, kind="ExternalInput").ap()
    selfT1 = nc.dram_tensor("selfT1", [IN_CH, S], bf16,
                            kind="ExternalInput").ap()
    invdeg = nc.dram_tensor("invdeg", [P, T], f32, kind="ExternalInput").ap()
    maskf = nc.dram_tensor("maskf", [P, T], f32, kind="ExternalInput").ap()
    localgf = nc.dram_tensor("localgf", [P, T], f32, kind="ExternalInput").ap()
    iota = nc.dram_tensor("iota", [P, POOL_SLOTS], f32,
                          kind="ExternalInput").ap()
    identf = nc.dram_tensor("identf", [P, P], f32, kind="ExternalInput").ap()
    identb = nc.dram_tensor("identb", [P, P], bf16, kind="ExternalInput").ap()
    w1lT = nc.dram_tensor("w1lT", [IN_CH, HID], f32, kind="ExternalInput").ap()
    w1rT = nc.dram_tensor("w1rT", [IN_CH, HID], bf16,
                          kind="ExternalInput").ap()
    b1rep = nc.dram_tensor("b1rep", [P, HID], f32, kind="ExternalInput").ap()
    w2lT = nc.dram_tensor("w2lT", [HID, HID], f32, kind="ExternalInput").ap()
    w2rT = nc.dram_tensor("w2rT", [HID, HID], bf16, kind="ExternalInput").ap()
    b2rep = nc.dram_tensor("b2rep", [P, HID], f32, kind="ExternalInput").ap()
    pool = nc.dram_tensor("pool", [POOL_SLOTS, HID], f32,
                          kind="ExternalOutput").ap()

    rg = [list(range(n_cores))]
    with tile.TileContext(nc) as tc:
        with tc.tile_pool(name="dramp", bufs=1, space="DRAM") as dpool, \
             tc.tile_pool(name="resident", bufs=1) as rpool:
            x_shard = dpool.tile([S, IN_CH], bf16, tag="x_shard")
            x_full = dpool.tile([AS, IN_CH], bf16, addr_space="Shared",
                                tag="x_full")
            h1_shard = dpool.tile([S, HID], bf16, tag="h1_shard")
            h1_full = dpool.tile([AS, HID], bf16, addr_space="Shared",
                                 tag="h1_full")

            # phase 0: AllGather x shards into the full (permuted) table
            nc.gpsimd.dma_start(x_shard[:], xs[:])
            if no_cc:
                nc.gpsimd.dma_start(x_full[:S, :], x_shard[:])
            else:
                nc.gpsimd.collective_compute(
                    "AllGather", mybir.AluOpType.bypass, replica_groups=rg,
                    ins=[x_shard[:].opt()], outs=[x_full[:].opt()])

            nbr_sb = rpool.tile([P, C], i32, tag="nbr")
            nc.sync.dma_start(nbr_sb[:], nbr[:])
            selfT1_sb = rpool.tile([IN_CH, S], bf16, tag="selfT1")
            nc.sync.dma_start(selfT1_sb[:], selfT1[:])
            invdeg_sb = rpool.tile([P, T], f32, tag="invdeg")
            nc.sync.dma_start(invdeg_sb[:], invdeg[:])
            maskf_sb = rpool.tile([P, T], f32, tag="maskf")
            nc.sync.dma_start(maskf_sb[:], maskf[:])
            localgf_sb = rpool.tile([P, T], f32, tag="localgf")
            nc.sync.dma_start(localgf_sb[:], localgf[:])
            iota_sb = rpool.tile([P, POOL_SLOTS], f32, tag="iota")
            nc.sync.dma_start(iota_sb[:], iota[:])
            identf_sb = rpool.tile([P, P], f32, tag="identf")
            nc.sync.dma_start(identf_sb[:], identf[:])
            identb_sb = rpool.tile([P, P], bf16, tag="identb")
            nc.sync.dma_start(identb_sb[:], identb[:])
            w1lT_sb = rpool.tile([IN_CH, HID], f32, tag="w1lT")
            nc.sync.dma_start(w1lT_sb[:], w1lT[:])
            w1rT_sb = rpool.tile([IN_CH, HID], bf16, tag="w1rT")
            nc.sync.dma_start(w1rT_sb[:], w1rT[:])
            b1rep_sb = rpool.tile([P, HID], f32, tag="b1rep")
            nc.sync.dma_start(b1rep_sb[:], b1rep[:])
            w2lT_sb = rpool.tile([HID, HID], f32, tag="w2lT")
            nc.sync.dma_start(w2lT_sb[:], w2lT[:])
            w2rT_sb = rpool.tile([HID, HID], bf16, tag="w2rT")
            nc.sync.dma_start(w2rT_sb[:], w2rT[:])
            b2rep_sb = rpool.tile([P, HID], f32, tag="b2rep")
            nc.sync.dma_start(b2rep_sb[:], b2rep[:])
            selfT2_sb = rpool.tile([HID, S], bf16, tag="selfT2")
            acc_sb = rpool.tile([P, POOL_CHUNKS * HID], f32, tag="acc")
            nc.vector.memset(acc_sb[:], 0.0)

            # ---------------- phase 1: layer 1 over x_full
            with tc.tile_pool(name="gather1", bufs=3) as gpool, \
                 tc.tile_pool(name="work1", bufs=3) as wpool, \
                 tc.tile_pool(name="ps_t1", bufs=2, space="PSUM") as ps_t, \
                 tc.tile_pool(name="ps_h1", bufs=2, space="PSUM") as ps_h, \
                 tc.tile_pool(name="ps_z1", bufs=2, space="PSUM") as ps_z:
                for t in range(T):
                    kt = 1 if p1_k1 else K[t]
                    g = gpool.tile([P, kt * IN_CH], bf16, tag="g")
                    for k in range(kt):
                        _q(nc.gpsimd.indirect_dma_start(
                            out=g[:, k * IN_CH:(k + 1) * IN_CH],
                            out_offset=None, in_=x_full[:],
                            in_offset=bass.IndirectOffsetOnAxis(
                                ap=nbr_sb[:, offs[t] + k:offs[t] + k + 1],
                                axis=0)))
                    agg = wpool.tile([P, IN_CH], f32, tag="agg")
                    nc.vector.tensor_reduce(
                        agg[:], g[:].rearrange("p (k f) -> p f k", k=kt),
                        axis=mybir.AxisListType.X, op=mybir.AluOpType.add)
                    nc.vector.tensor_scalar_mul(agg[:], agg[:],
                                                invdeg_sb[:, t:t + 1])
                    aggT_ps = ps_t.tile([IN_CH, P], f32, tag="aggT")
                    nc.tensor.transpose(aggT_ps[:], agg[:], identf_sb[:])
                    aggT = wpool.tile([IN_CH, P], f32, tag="aggTs")
                    nc.vector.tensor_copy(aggT[:], aggT_ps[:])
                    z_ps = ps_z.tile([P, HID], f32, tag="z")
                    nc.tensor.matmul(z_ps[:], lhsT=aggT[:], rhs=w1lT_sb[:],
                                     start=True, stop=False)
                    nc.tensor.matmul(z_ps[:],
                                     lhsT=selfT1_sb[:, t * P:(t + 1) * P],
                                     rhs=w1rT_sb[:], start=False, stop=True)
                    zb = wpool.tile([P, HID], f32, tag="zb")
                    nc.vector.tensor_tensor(zb[:], z_ps[:], b1rep_sb[:],
                                            op=mybir.AluOpType.add)
                    nc.vector.tensor_scalar_mul(zb[:], zb[:],
                                                maskf_sb[:, t:t + 1])
                    h1t = wpool.tile([P, HID], bf16, tag="h1t")
                    nc.scalar.activation(h1t[:], zb[:],
                                         mybir.ActivationFunctionType.Relu)
                    nc.sync.dma_start(h1_shard[t * P:(t + 1) * P, :], h1t[:])
                    h1T_ps = ps_h.tile([HID, P], bf16, tag="h1T")
                    nc.tensor.transpose(h1T_ps[:], h1t[:], identb_sb[:])
                    nc.vector.tensor_copy(selfT2_sb[:, t * P:(t + 1) * P],
                                          h1T_ps[:])

            # AllGather h1 across cores
            if no_cc:
                nc.gpsimd.dma_start(h1_full[:S, :], h1_shard[:])
            else:
                nc.gpsimd.collective_compute(
                    "AllGather", mybir.AluOpType.bypass, replica_groups=rg,
                    ins=[h1_shard[:].opt()], outs=[h1_full[:].opt()])

            # ---------------- phase 2: layer 2 over h1_full + pooling
            with tc.tile_pool(name="gather2", bufs=3) as gpool, \
                 tc.tile_pool(name="work2", bufs=3) as wpool, \
                 tc.tile_pool(name="ps_t2", bufs=2, space="PSUM") as ps_t, \
                 tc.tile_pool(name="ps_z2", bufs=2, space="PSUM") as ps_z, \
                 tc.tile_pool(name="ps_p2", bufs=2, space="PSUM") as ps_p:
                for t in range(T):
                    kt = 1 if p2_k1 else K[t]
                    g = gpool.tile([P, kt * HID], bf16, tag="g2")
                    for k in range(kt):
                        _q(nc.gpsimd.indirect_dma_start(
                            out=g[:, k * HID:(k + 1) * HID],
                            out_offset=None, in_=h1_full[:],
                            in_offset=bass.IndirectOffsetOnAxis(
                                ap=nbr_sb[:, offs[t] + k:offs[t] + k + 1],
                                axis=0)))
                    agg = wpool.tile([P, HID], f32, tag="agg2")
                    nc.vector.tensor_reduce(
                        agg[:], g[:].rearrange("p (k f) -> p f k", k=kt),
                        axis=mybir.AxisListType.X, op=mybir.AluOpType.add)
                    nc.vector.tensor_scalar_mul(agg[:], agg[:],
                                                invdeg_sb[:, t:t + 1])
                    aggT_ps = ps_t.tile([HID, P], f32, tag="aggT2")
                    nc.tensor.transpose(aggT_ps[:], agg[:], identf_sb[:])
                    aggT = wpool.tile([HID, P], f32, tag="aggTs2")
                    nc.vector.tensor_copy(aggT[:], aggT_ps[:])
                    z_ps = ps_z.tile([P, HID], f32, tag="z2")
                    nc.tensor.matmul(z_ps[:], lhsT=aggT[:], rhs=w2lT_sb[:],
                                     start=True, stop=False)
                    nc.tensor.matmul(z_ps[:],
                                     lhsT=selfT2_sb[:, t * P:(t + 1) * P],
                                     rhs=w2rT_sb[:], start=False, stop=True)
                    zb = wpool.tile([P, HID], f32, tag="zb2")
                    nc.vector.tensor_tensor(zb[:], z_ps[:], b2rep_sb[:],
                                            op=mybir.AluOpType.add)
                    h2t = wpool.tile([P, HID], bf16, tag="h2t")
                    nc.scalar.activation(h2t[:], zb[:],
                                         mybir.ActivationFunctionType.Relu)
                    if no_pool:
                        continue
                    # one-hot pooling: ind[p, s] = (localg[p] == s)
                    ind = wpool.tile([P, POOL_SLOTS], bf16, tag="ind")
                    nc.vector.tensor_scalar(
                        ind[:], iota_sb[:], localgf_sb[:, t:t + 1], None,
                        op0=mybir.AluOpType.is_equal)
                    for ch in range(POOL_CHUNKS):
                        pp = ps_p.tile([P, HID], f32, tag="pp")
                        nc.tensor.matmul(
                            pp[:], lhsT=ind[:, ch * P:(ch + 1) * P],
                            rhs=h2t[:], start=True, stop=True)
                        a = acc_sb[:, ch * HID:(ch + 1) * HID]
                        nc.vector.tensor_tensor(a, a, pp[:],
                                                op=mybir.AluOpType.add)

            for ch in range(POOL_CHUNKS):
                nc.sync.dma_start(pool[ch * P:(ch + 1) * P, :],
                                  acc_sb[:, ch * HID:(ch + 1) * HID])

    nc.compile()
    return nc


# ---------------------------------------------------------------- jit runner
class _PjrtRunner:
    """Persistent jit(shard_map) executor for a compiled Bass module.

    Same _bass_exec_p machinery run_bass_kernel_spmd uses under axon, but
    the jitted callable and device-resident inputs persist across calls so
    steady-state executions can be timed without re-trace/re-compile or
    host->device traffic.
    """

    def __init__(self, nc, n_cores):
        import jax
        from jax.experimental.shard_map import shard_map
        from jax.sharding import Mesh, NamedSharding, PartitionSpec
        from concourse import bass2jax

        bass2jax.install_neuronx_cc_hook()
        self.jax = jax
        self.nc = nc
        self.n_cores = n_cores
        partition_name = (nc.partition_id_tensor.name
                          if nc.partition_id_tensor else None)
        in_names, out_names, out_avals, zero_shapes = [], [], [], []
        for alloc in nc.m.functions[0].allocations:
            if not isinstance(alloc, mybir.MemoryLocationSet):
                continue
            name = alloc.memorylocations[0].name
            if alloc.kind == "ExternalInput":
                if name != partition_name:
                    in_names.append(name)
            elif alloc.kind == "ExternalOutput":
                shape = tuple(alloc.tensor_shape)
                dtype = mybir.dt.np(alloc.dtype)
                out_names.append(name)
                out_avals.append(jax.core.ShapedArray(shape, dtype))
                zero_shapes.append((shape, dtype))
        self.in_names = list(in_names)
        self.out_names = out_names
        self.zero_shapes = zero_shapes
        n_params = len(in_names)
        n_outs = len(out_names)
        all_names = in_names + out_names
        if partition_name is not None:
            all_names.append(partition_name)
        donate = tuple(range(n_params, n_params + n_outs))

        def _body(*args):
            operands = list(args)
            if partition_name is not None:
                operands.append(bass2jax.partition_id_tensor())
            outs = bass2jax._bass_exec_p.bind(
                *operands,
                out_avals=tuple(out_avals),
                in_names=tuple(all_names),
                out_names=tuple(out_names),
                lowering_input_output_aliases=(),
                sim_require_finite=True,
                sim_require_nnan=True,
                nc=nc,
            )
            return tuple(outs)

        devices = jax.devices()[:n_cores]
        assert len(devices) == n_cores
        self.mesh = Mesh(np.asarray(devices), ("core",))
        self.sharding = NamedSharding(self.mesh, PartitionSpec("core"))
        in_specs = (PartitionSpec("core"),) * (n_params + n_outs)
        out_specs = (PartitionSpec("core"),) * n_outs
        self._fn = jax.jit(
            shard_map(_body, mesh=self.mesh, in_specs=in_specs,
                      out_specs=out_specs, check_rep=False),
            donate_argnums=donate, keep_unused=True)
        self._dev_inputs = None

    def put_inputs(self, in_maps):
        """Concatenate per-core inputs on axis 0 and place on device."""
        jax = self.jax
        self._dev_inputs = []
        for name in self.in_names:
            arr = np.concatenate([np.asarray(m[name]) for m in in_maps],
                                 axis=0)
            self._dev_inputs.append(
                jax.device_put(arr, self.sharding))
        jax.block_until_ready(self._dev_inputs)

    def _zeros(self):
        jax = self.jax
        zs = [jax.device_put(
                  np.zeros((self.n_cores * s[0], *s[1:]), d), self.sharding)
              for s, d in self.zero_shapes]
        jax.block_until_ready(zs)
        return zs

    def run(self, zeros=None):
        jax = self.jax
        if zeros is None:
            zeros = self._zeros()
        outs = self._fn(*self._dev_inputs, *zeros)
        jax.block_until_ready(outs)
        return outs

    def results(self, outs):
        per_core = []
        for c in range(self.n_cores):
            m = {}
            for i, name in enumerate(self.out_names):
                shape, _ = self.zero_shapes[i]
                m[name] = np.asarray(outs[i]).reshape(
                    self.n_cores, *shape)[c]
            per_core.append(m)
        return per_core

    def time_runs(self, reps=3):
        zero_sets = [self._zeros() for _ in range(reps)]
        times = []
        for zs in zero_sets:
            t0 = time.perf_counter()
            outs = self._fn(*self._dev_inputs, *zs)
            self.jax.block_until_ready(outs)
            times.append(time.perf_counter() - t0)
        return times

    def time_pipelined(self, reps=10):
        """Issue `reps` executions back-to-back (async dispatch), block once.

        Per-run time = total / reps: dispatch latency overlaps execution, so
        this converges to the device execution+collective time per run.
        """
        zero_sets = [self._zeros() for _ in range(reps)]
        all_outs = []
        t0 = time.perf_counter()
        for zs in zero_sets:
            all_outs.append(self._fn(*self._dev_inputs, *zs))
        self.jax.block_until_ready(all_outs)
        return (time.perf_counter() - t0) / reps


# -------------------------------------------------------------------- kernel
def _kernel_impl(x, edge_index, batch, W1_l, b1, W1_r, W2_l, b2, W2_r,
                 W_lin, b_lin, n_nodes, n_graphs, n_cores, _timing=None,
                 mode="eg"):
    x = np.asarray(x, dtype=np.float32)
    W1_l = np.asarray(W1_l, np.float32)
    W1_r = np.asarray(W1_r, np.float32)
    W2_l = np.asarray(W2_l, np.float32)
    W2_r = np.asarray(W2_r, np.float32)
    b1 = np.asarray(b1, np.float32)
    b2 = np.asarray(b2, np.float32)
    W_lin = np.asarray(W_lin, np.float32)
    b_lin = np.asarray(b_lin, np.float32)

    iota = np.broadcast_to(
        np.arange(POOL_SLOTS, dtype=np.float32), (P, POOL_SLOTS)).copy()
    identb = np.eye(P).astype(bfl)
    b2rep = np.broadcast_to(b2, (P, HID)).copy()

    t0 = time.time()
    if mode == "eg":
        pp = _prep_eg(edge_index, batch, n_nodes, n_graphs, n_cores)
        T, S = pp["T"], pp["S"]
        t_prep = time.time() - t0

        t0 = time.time()
        nc = _build_eg(T, S, pp["GN"], pp["NB"], pp["POS"], pp["pos_base"],
                       pp["segs"], n_cores)
        t_build = time.time() - t0

        w1big = np.zeros((2 * IN_CH + 1, HID), np.float32)
        w1big[:IN_CH] = W1_l.T
        w1big[IN_CH:2 * IN_CH] = W1_r.T
        w1big[2 * IN_CH] = b1
        w2big = np.concatenate([W2_l.T, W2_r.T], axis=0)
        in_maps = []
        for ci in pp["cores"]:
            xs = np.zeros((S, 64), bfl)
            xs[:ci["n"], :IN_CH] = x[ci["ids"]]
            in_maps.append(dict(
                xs=xs,
                selfT1=np.ascontiguousarray(xs[:, :IN_CH].T),
                idx=ci["idx"], dinv=ci["dinv"],
                localgf=ci["localgf"], iota=iota, identb=identb,
                w1big=w1big.astype(bfl), w2big=w2big.astype(bfl),
                b2rep=b2rep))
    else:
        pp = _prep(edge_index, batch, n_nodes, n_graphs, n_cores)
        T, S, K, offs, C = pp["T"], pp["S"], pp["K"], pp["offs"], pp["C"]
        t_prep = time.time() - t0

        t0 = time.time()
        nc = _build_fused(T, K, offs, C, S, n_cores)
        t_build = time.time() - t0

        identf = np.eye(P, dtype=np.float32)
        b1rep = np.broadcast_to(b1, (P, HID)).copy()
        in_maps = []
        for ci in pp["cores"]:
            xs = np.zeros((S, IN_CH), bfl)
            xs[:ci["n"]] = x[ci["ids"]]
            in_maps.append(dict(
                xs=xs, nbr=ci["nbr"],
                selfT1=np.ascontiguousarray(xs.T),
                invdeg=ci["invdeg"], maskf=ci["maskf"], localgf=ci["localgf"],
                iota=iota, identf=identf, identb=identb,
                w1lT=np.ascontiguousarray(W1_l.T),
                w1rT=np.ascontiguousarray(W1_r.T).astype(bfl),
                b1rep=b1rep,
                w2lT=np.ascontiguousarray(W2_l.T),
                w2rT=np.ascontiguousarray(W2_r.T).astype(bfl),
                b2rep=b2rep))

    t0 = time.time()
    runner = _PjrtRunner(nc, n_cores)
    runner.put_inputs(in_maps)
    t_put = time.time() - t0

    t0 = time.time()
    outs = runner.run()  # first call compiles (NEFF via neuronx_cc hook)
    t_first = time.time() - t0
    res = runner.results(outs)

    # host: divide pooled sums by graph node counts; final linear
    pool = np.zeros((n_graphs, HID), np.float32)
    for c, (ci, r) in enumerate(zip(pp["cores"], res)):
        ng = ci["ghi"] - ci["glo"]
        rp = r["pool"]
        if rp.shape[0] == HID:  # eg kernel outputs [HID, POOL_SLOTS]
            rp = np.ascontiguousarray(rp.T)
        pool[ci["glo"]:ci["ghi"]] = rp[:ng]
    gcnt = np.maximum(pp["gcnt"], 1).astype(np.float32)
    pooled = pool / gcnt[:, None]
    out = (pooled @ W_lin.T + b_lin).astype(np.float32)

    if _timing is not None:
        times = runner.time_runs(reps=2)
        tot40 = runner.time_pipelined(reps=40) * 40
        tot80 = runner.time_pipelined(reps=80) * 80
        _timing.update(prep=t_prep, build=t_build, put=t_put,
                       first=t_first, reps=times,
                       piped80=tot80 / 80, marginal=(tot80 - tot40) / 40,
                       exec_ns=tot80 / 80 * 1e9)
    return out


def kernel(x, edge_index, batch, W1_l, b1, W1_r, W2_l, b2, W2_r, W_lin,
           b_lin, _timing=None):
    return _kernel_impl(x, edge_index, batch, W1_l, b1, W1_r, W2_l, b2,
                        W2_r, W_lin, b_lin, N_NODES, N_GRAPHS, N_CORES,
                        _timing=_timing)
